# revision 1
# baseline (speedup 1.0000x reference)
"""Trainium2 Bass kernel for nn_Attention_45999099740384.

GQA attention over 8 independent packed sequences (block-diagonal mask with
equal blocks). Sharding: pure data-parallel over the 8 blocks — one block of
S=1024 tokens per NeuronCore, weights replicated, zero collectives (blocks
are fully independent; the output rows of block b depend only on x rows of
block b).

Per-core pipeline (bf16 TensorEngine, fp32 PSUM):
  1. natural-layout q/k/v projections from host-pre-transposed xT
  2. RoPE on VectorE (host-permuted wq/wk columns put the even/odd rotation
     pairs into contiguous 64-wide halves of each head)
  3. PE-transpose q/k into [head_dim, T] layout
  4. scores computed transposed: ST[s,q] = kT.T @ qT -> ScalarE exp ->
     P^T tiles in SBUF; P@V then needs NO transpose of P
     (out^T[d,q] = sum_s v[s,d] * PT[s,q], lhsT = v in natural layout)
  5. softmax row-sums via a ones[128,128] stationary matmul; reciprocal on a
     single partition + GpSimd partition_broadcast; normalization deferred
     to after P@V
  6. wo matmul from the transposed attention output (already in the right
     layout), fp32 output.

Weights are shipped in a [n_col_chunks, 2, 128, 16, 512] layout so every
DMA slab is contiguous per partition, and large loads are split into
multiple dma_start instructions to spread across DMA queues.
"""

import numpy as np
import ml_dtypes

import concourse.bass as bass
import concourse.mybir as mybir
import concourse.tile as tile
from concourse import bacc
from concourse.bass_utils import run_bass_kernel_spmd
from concourse.masks import make_identity
from concourse import bass_isa

# problem constants (hardcoded per task instructions)
DIM = 4096
N_HEADS = 32
HEAD_DIM = 128
N_KV = 8
REP = 4
B = 8
S = 1024
T = B * S

P = 128                  # SBUF partitions
KC = DIM // P            # 32 contraction chunks of 128
KH = KC // 2             # 16 chunks per half-slab
TT = S // P              # 8 token tiles per core
NCH = 512                # matmul moving free dim
SCALE = HEAD_DIM ** -0.5

F32 = mybir.dt.float32
BF16 = mybir.dt.bfloat16

_CACHE = {}


def build_nc():
    nc = bacc.Bacc("TRN2", target_bir_lowering=False, debug=False, num_devices=8)

    # per-core DRAM parameters (bf16, layouts prepared host-side)
    xt_d = nc.dram_tensor("xt", [P, KC, S], BF16, kind="ExternalInput")
    cs_d = nc.dram_tensor("cs", [P, TT, 64], F32, kind="ExternalInput")
    sn_d = nc.dram_tensor("sn", [P, TT, 64], F32, kind="ExternalInput")
    # weights: [n_col_chunks, 2 halves, 128 p, 16 kc, 512 c]
    wq_d = nc.dram_tensor("wq", [8, 2, P, KH, NCH], BF16, kind="ExternalInput")
    wk_d = nc.dram_tensor("wk", [2, 2, P, KH, NCH], BF16, kind="ExternalInput")
    wv_d = nc.dram_tensor("wv", [2, 2, P, KH, NCH], BF16, kind="ExternalInput")
    wo_d = nc.dram_tensor("wo", [8, 2, P, KH, NCH], BF16, kind="ExternalInput")
    out_d = nc.dram_tensor("out", [S, DIM], F32, kind="ExternalOutput")
    # attention-output bounce, [qc, d, h, t] so both sides are contiguous
    otb_d = nc.dram_tensor("otb", [2, P, N_HEADS, NCH], BF16)

    with tile.TileContext(nc) as tc:
        with (
            tc.tile_pool(name="const", bufs=1) as const,
            tc.tile_pool(name="wslab", bufs=6) as wslab_pool,
            tc.tile_pool(name="qtg", bufs=2) as qtg_pool,
            tc.tile_pool(name="nat", bufs=3) as nat_pool,
            tc.tile_pool(name="pt", bufs=2) as pt_pool,
            tc.tile_pool(name="scr", bufs=5) as scr_pool,
            tc.tile_pool(name="rcb", bufs=4) as rcb_pool,
            tc.tile_pool(name="otile", bufs=2) as ot_pool,
            tc.tile_pool(name="outp", bufs=3) as out_pool,
            tc.tile_pool(name="psmm", bufs=2, space="PSUM") as ps_pool,
            tc.tile_pool(name="psor", bufs=2, space="PSUM") as or_pool,
            tc.tile_pool(name="psst", bufs=2, space="PSUM") as st_pool,
            tc.tile_pool(name="pstp", bufs=2, space="PSUM") as tp_pool,
        ):
            # ---- constants ----
            ones_t = const.tile([P, P], BF16)
            nc.vector.memset(ones_t[:], 1.0)
            ident = const.tile([P, P], BF16)
            make_identity(nc, ident[:])
            kvres_cm = tc.tile_pool(name="kvres", bufs=1)
            kvres = kvres_cm.__enter__()
            kT = kvres.tile([P, N_KV, S], BF16)      # [d, kv, s]
            vN = kvres.tile([P, TT, N_KV * HEAD_DIM], BF16)  # [s_in, s_tile, kv*d]

            def load_w_halves(w_dram, cc, tag, splits=1):
                """Stream one 512-col weight chunk as 4 contiguous
                quarter-slabs (one dma_start each, separate queues)."""
                quarters = []
                for q in range(4):
                    sl = wslab_pool.tile([P, KH // 2, NCH], BF16, tag=tag)
                    kq = KH // 2
                    for j in range(splits):
                        nc.sync.dma_start(
                            out=sl[:, j * (kq // splits):(j + 1) * (kq // splits), :],
                            in_=w_dram.ap()[cc, q // 2, :,
                                            (q % 2) * kq + j * (kq // splits):
                                            (q % 2) * kq + (j + 1) * (kq // splits), :],
                        )
                    quarters.append(sl)
                return quarters

            xres_cm = tc.tile_pool(name="xres", bufs=1)
            xres = xres_cm.__enter__()
            xt = xres.tile([P, KC, S], BF16)
            for i in range(4):  # first t-tile, 4-way parallel
                nc.sync.dma_start(
                    out=xt[:, i * 8:(i + 1) * 8, 0:128],
                    in_=xt_d.ap()[:, i * 8:(i + 1) * 8, 0:128],
                )
            wk_first = load_w_halves(wk_d, 0, tag="wsl", splits=2)
            for t0, t1 in ((128, 384), (384, 768), (768, 1024)):
                nc.sync.dma_start(
                    out=xt[:, :, t0:t1],
                    in_=xt_d.ap()[:, :, t0:t1],
                )
            cs = const.tile([P, TT, 64], F32)
            nc.sync.dma_start(out=cs[:], in_=cs_d.ap())
            sn = const.tile([P, TT, 64], F32)
            nc.sync.dma_start(out=sn[:], in_=sn_d.ap())

            def proj_psum(halves, tt):
                """psum[128 t, 512 cols] = x_tile @ W[:, cols]"""
                ps = ps_pool.tile([P, NCH], F32, tag="mm")
                for kc in range(KC):
                    nc.tensor.matmul(
                        ps[:],
                        lhsT=xt[:, kc, tt * P:(tt + 1) * P],
                        rhs=halves[kc // (KH // 2)][:, kc % (KH // 2), :],
                        start=(kc == 0),
                        stop=(kc == KC - 1),
                    )
                return ps

            def rope(ps, tt, nat):
                """RoPE on a [128 t, 4 heads x (64 even | 64 odd)] psum tile,
                writing bf16 into `nat` (same layout)."""
                v3 = ps[:].rearrange("p (h d) -> p h d", h=4)
                n3 = nat[:].rearrange("p (h d) -> p h d", h=4)
                qe = v3[:, :, 0:64]
                qo = v3[:, :, 64:128]
                cs_b = cs[:, tt, None, :].to_broadcast((P, 4, 64))
                sn_b = sn[:, tt, None, :].to_broadcast((P, 4, 64))
                s1 = scr_pool.tile([P, 4, 64], F32, tag="scr")
                s2 = scr_pool.tile([P, 4, 64], F32, tag="scr")
                nc.vector.tensor_tensor(s1[:], qe, cs_b, mybir.AluOpType.mult)
                nc.vector.tensor_tensor(s2[:], qo, sn_b, mybir.AluOpType.mult)
                nc.vector.tensor_tensor(n3[:, :, 0:64], s1[:], s2[:],
                                        mybir.AluOpType.subtract)
                s3 = scr_pool.tile([P, 4, 64], F32, tag="scr")
                s4 = scr_pool.tile([P, 4, 64], F32, tag="scr")
                nc.vector.tensor_tensor(s3[:], qe, sn_b, mybir.AluOpType.mult)
                nc.vector.tensor_tensor(s4[:], qo, cs_b, mybir.AluOpType.mult)
                nc.vector.tensor_tensor(n3[:, :, 64:128], s3[:], s4[:],
                                        mybir.AluOpType.add)

            def transpose_heads(nat, tt, dest, h0):
                """PE-transpose the four [128 t, 128 d] head blocks of `nat`
                into dest[:, h0:h0+4, tt*128:(tt+1)*128] ([d, t] layout)."""
                tp = tp_pool.tile([P, 4, P], BF16, tag="tp")
                for i in range(4):
                    nc.tensor.transpose(tp[:, i, :], nat[:, i * P:(i + 1) * P],
                                        ident[:])
                nc.any.tensor_copy(
                    out=dest[:, h0:h0 + 4, tt * P:(tt + 1) * P], in_=tp[:]
                )

            # ---- k projection -> RoPE -> kT ----
            for cc in range(2):  # 4 kv heads per column chunk
                halves = wk_first if cc == 0 else load_w_halves(wk_d, cc, tag="wsl")
                for tt in range(TT):
                    ps = proj_psum(halves, tt)
                    nat = nat_pool.tile([P, NCH], BF16, tag="nat")
                    rope(ps, tt, nat)
                    transpose_heads(nat, tt, kT, cc * 4)

            # ---- v projection (natural layout, no RoPE) ----
            for cc in range(2):
                halves = load_w_halves(wv_d, cc, tag="wsl")
                for tt in range(TT):
                    ps = proj_psum(halves, tt)
                    nc.any.tensor_copy(
                        out=vN[:, tt, cc * NCH:(cc + 1) * NCH], in_=ps[:]
                    )

            # ---- per kv-group: q projection + attention ----
            def q_project(g, halves):
                qT = qtg_pool.tile([P, REP, S], BF16, tag="qtg")  # [d, rep, t]
                for tt in range(TT):
                    ps = proj_psum(halves, tt)
                    nat = nat_pool.tile([P, NCH], BF16, tag="nat")
                    rope(ps, tt, nat)
                    transpose_heads(nat, tt, qT, 0)
                return qT

            def attention_group(g, qT):
                for r in range(REP):
                    for qc in range(2):
                        pt = pt_pool.tile([P, TT, NCH], BF16, tag="pt")
                        for st in range(TT):
                            sps = st_pool.tile([P, NCH], F32, tag="st")
                            nc.tensor.matmul(
                                sps[:],
                                lhsT=kT[:, g, st * P:(st + 1) * P],
                                rhs=qT[:, r, qc * NCH:(qc + 1) * NCH],
                                start=True,
                                stop=True,
                            )
                            nc.scalar.activation(
                                pt[:, st, :], sps[:],
                                mybir.ActivationFunctionType.Exp,
                                scale=SCALE,
                            )
                        ops = or_pool.tile([P, NCH], F32, tag="ors")
                        rps = or_pool.tile([P, NCH], F32, tag="ors")
                        for st in range(TT):
                            nc.tensor.matmul(
                                ops[:],
                                lhsT=vN[:, st, g * P:(g + 1) * P],
                                rhs=pt[:, st, :],
                                start=(st == 0),
                                stop=(st == TT - 1),
                            )
                        for st in range(TT):
                            nc.tensor.matmul(
                                rps[:],
                                lhsT=ones_t[:],
                                rhs=pt[:, st, :],
                                start=(st == 0),
                                stop=(st == TT - 1),
                            )
                        lnt = rcb_pool.tile([P, NCH], F32, tag="rcb")
                        nc.scalar.activation(lnt[:], rps[:],
                                             mybir.ActivationFunctionType.Ln)
                        rcb = rcb_pool.tile([P, NCH], F32, tag="rcb")
                        nc.scalar.activation(rcb[:], lnt[:],
                                             mybir.ActivationFunctionType.Exp,
                                             scale=-1.0)
                        ot = ot_pool.tile([P, NCH], BF16, tag="ot")
                        nc.vector.tensor_tensor(ot[:], ops[:], rcb[:],
                                                mybir.AluOpType.mult)
                        nc.sync.dma_start(
                            out=otb_d.ap()[qc, :, g * REP + r, :],
                            in_=ot[:],
                        )

            halves_g = load_w_halves(wq_d, 0, tag="wsl")
            for g in range(N_KV - 1):
                qT = q_project(g, halves_g)
                # prefetch next group's weights ahead of this group's attention
                halves_g = load_w_halves(wq_d, g + 1, tag="wsl")
                attention_group(g, qT)
            qT7 = q_project(N_KV - 1, halves_g)

            # release x residency now (last q projection emitted); stage the
            # finished groups' attention output while group 7's attention runs
            xres_cm.__exit__(None, None, None)
            ores_cm = tc.tile_pool(name="ores", bufs=1)
            ores = ores_cm.__enter__()
            ot_all = ores.tile([P, N_HEADS, S], BF16)  # [d, h, t]
            for qc in range(2):
                for j in range(7):
                    nc.sync.dma_start(
                        out=ot_all[:, j * 4:(j + 1) * 4, qc * NCH:(qc + 1) * NCH],
                        in_=otb_d.ap()[qc, :, j * 4:(j + 1) * 4, :],
                    )
            attention_group(N_KV - 1, qT7)
            for qc in range(2):
                nc.sync.dma_start(
                    out=ot_all[:, 28:32, qc * NCH:(qc + 1) * NCH],
                    in_=otb_d.ap()[qc, :, 28:32, :],
                )
            for nc5 in range(DIM // NCH):
                halves = load_w_halves(wo_d, nc5, tag="wsl")
                for tt in range(TT):
                    ps = ps_pool.tile([P, NCH], F32, tag="mm")
                    for h in range(N_HEADS):
                        nc.tensor.matmul(
                            ps[:],
                            lhsT=ot_all[:, h, tt * P:(tt + 1) * P],
                            rhs=halves[h // (KH // 2)][:, h % (KH // 2), :],
                            start=(h == 0),
                            stop=(h == N_HEADS - 1),
                        )
                    outt = out_pool.tile([P, NCH], F32, tag="outp")
                    nc.vector.tensor_copy(out=outt[:], in_=ps[:])
                    nc.sync.dma_start(
                        out=out_d.ap()[tt * P:(tt + 1) * P,
                                       nc5 * NCH:(nc5 + 1) * NCH],
                        in_=outt[:],
                    )
            ores_cm.__exit__(None, None, None)
            kvres_cm.__exit__(None, None, None)

    nc.compile()
    return nc


# host-side input preparation -------------------------------------------------

_ROPE_PERM = np.concatenate([np.arange(0, HEAD_DIM, 2), np.arange(1, HEAD_DIM, 2)])


def _permute_heads(w, n_heads):
    """Permute columns within each head so rotation pairs become
    contiguous (even | odd) halves."""
    w = w.reshape(w.shape[0], n_heads, HEAD_DIM)
    return w[:, :, _ROPE_PERM].reshape(w.shape[0], n_heads * HEAD_DIM)


def _w_layout(w):
    """[DIM, C] f32 -> [C/512, 2, 128, 16, 512] bf16 slab layout."""
    C = w.shape[1]
    wl = w.reshape(2, KH, P, C // NCH, NCH).transpose(3, 0, 2, 1, 4)
    return np.ascontiguousarray(wl).astype(ml_dtypes.bfloat16)


def _prep_shared(cos, sin, wq, wk, wv, wo):
    wq_p = _permute_heads(np.asarray(wq, dtype=np.float32), N_HEADS)
    wk_p = _permute_heads(np.asarray(wk, dtype=np.float32), N_KV)
    wq_l = _w_layout(wq_p)
    wk_l = _w_layout(wk_p)
    wv_l = _w_layout(np.asarray(wv, dtype=np.float32))
    wo_l = _w_layout(np.asarray(wo, dtype=np.float32))
    # positions restart at 0 per block, so block 0's table serves all cores
    cs_l = np.ascontiguousarray(
        np.asarray(cos[:S], dtype=np.float32).reshape(TT, P, 64).transpose(1, 0, 2)
    )
    sn_l = np.ascontiguousarray(
        np.asarray(sin[:S], dtype=np.float32).reshape(TT, P, 64).transpose(1, 0, 2)
    )
    return cs_l, sn_l, wq_l, wk_l, wv_l, wo_l


def _prep_x_block(xb):
    """x block [S, DIM] f32 -> xt [128, KC, S] bf16 (transposed)."""
    bf = ml_dtypes.bfloat16
    xtb = xb.T.reshape(KC, P, S).transpose(1, 0, 2)
    return np.ascontiguousarray(xtb).astype(bf)


def kernel(x, cos, sin, wq, wk, wv, wo):
    if "nc" not in _CACHE:
        _CACHE["nc"] = build_nc()
    nc = _CACHE["nc"]

    x = np.asarray(x, dtype=np.float32)
    cs_l, sn_l, wq_l, wk_l, wv_l, wo_l = _prep_shared(cos, sin, wq, wk, wv, wo)

    in_maps = []
    for b in range(B):
        in_maps.append({
            "xt": _prep_x_block(x[b * S:(b + 1) * S]),
            "cs": cs_l,
            "sn": sn_l,
            "wq": wq_l,
            "wk": wk_l,
            "wv": wv_l,
            "wo": wo_l,
        })
    _CACHE["last_in_maps"] = in_maps
    res = run_bass_kernel_spmd(nc, in_maps, core_ids=list(range(B)))
    _CACHE["last_results"] = res
    out = np.concatenate([res.results[b]["out"] for b in range(B)], axis=0)
    return out.astype(np.float32)



# revision 5
# speedup vs baseline: 1.0761x; 1.0761x over previous
"""Trainium2 Bass kernel for nn_Attention_45999099740384.

GQA attention over 8 independent packed sequences (block-diagonal mask with
equal blocks). Sharding: data-parallel over the 8 blocks - one block of
S=1024 tokens per NeuronCore, weights replicated, zero collectives.

Per-core pipeline (bf16 TensorEngine, fp32 PSUM):
  1. v projection in natural layout (xt stationary, wv moving).
  2. q/k projections TRANSPOSED (weight chunks stationary, xt moving) so
     q^T/k^T come out in [head_dim, t] layout directly - no PE transposes.
  3. RoPE applied in transposed layout on VectorE: host-permuted weight
     columns put rotation pairs into (even|odd) partition halves; the
     cross-partition half-swap is done with two 64-partition copies, then
     two mults against duplicated cos / sign-flipped sin tables and an add.
  4. scores computed transposed: ST[s,q] = kT.T @ qT -> ScalarE exp ->
     P^T tiles in SBUF; P@V needs no transpose of P.
  5. softmax row-sums via a ones[128,128] stationary matmul; reciprocal on
     VectorE (keeps ScalarE exp-only: a single activation table, no
     ACT_TABLE_LOAD churn); normalization deferred to after P@V.
  6. wo matmul from the transposed attention output.

Scheduling: the q projection of group g+1 is woven between the score
matmuls of group g's attention units so the PE never waits for ScalarE
exp; for the last group the leading wo matmul chains are woven in instead.
Attention outputs of groups 0-6 bounce through DRAM (SBUF cannot hold
ot_all while xt is still resident); group 7 writes ot_all directly.
"""

import numpy as np
import ml_dtypes

import concourse.bass as bass
import concourse.mybir as mybir
import concourse.tile as tile
from concourse import bacc
from concourse.bass_utils import run_bass_kernel_spmd

# problem constants (hardcoded per task instructions)
DIM = 4096
N_HEADS = 32
HEAD_DIM = 128
N_KV = 8
REP = 4
B = 8
S = 1024
T = B * S

P = 128                  # SBUF partitions
KC = DIM // P            # 32 contraction chunks of 128
KH = KC // 2             # 16 (w_layout half-chunk count)
TT = S // P              # 8 token tiles of 128
TC = S // 512            # 2 token chunks of 512
NCH = 512                # matmul moving free dim
SCALE = HEAD_DIM ** -0.5

F32 = mybir.dt.float32
BF16 = mybir.dt.bfloat16

_CACHE = {}


def build_nc():
    nc = bacc.Bacc("TRN2", target_bir_lowering=False, debug=False, num_devices=8)

    xt_d = nc.dram_tensor("xt", [P, KC, S], BF16, kind="ExternalInput")
    c2_d = nc.dram_tensor("c2", [P, S], F32, kind="ExternalInput")
    s2n_d = nc.dram_tensor("s2n", [P, S], F32, kind="ExternalInput")
    # wq: [g, h, k-part, kc, d] lhsT chunks (rope-permuted d columns)
    wq_d = nc.dram_tensor("wq", [N_KV, REP, P, KC, HEAD_DIM], BF16,
                          kind="ExternalInput")
    wk_d = nc.dram_tensor("wk", [N_KV, P, KC, HEAD_DIM], BF16,
                          kind="ExternalInput")
    # wv/wo: moving-operand slabs [chunk, half, 128, 16, 512]
    wv_d = nc.dram_tensor("wv", [2, 2, P, KH, NCH], BF16, kind="ExternalInput")
    wo_d = nc.dram_tensor("wo", [8, 2, P, KH, NCH], BF16, kind="ExternalInput")
    out_d = nc.dram_tensor("out", [S, DIM], F32, kind="ExternalOutput")
    # attention-output bounce for groups 0..6
    otb_d = nc.dram_tensor("otb", [2, P, 28, NCH], BF16)

    with tile.TileContext(nc) as tc:
        with (
            tc.tile_pool(name="const", bufs=1) as const,
            tc.tile_pool(name="wpool", bufs=1) as wpool,
            tc.tile_pool(name="qtg", bufs=2) as qtg_pool,
            tc.tile_pool(name="ptp", bufs=2) as pt_pool,
            tc.tile_pool(name="scr", bufs=3) as scr_pool,
            tc.tile_pool(name="rcbp", bufs=2) as rcb_pool,
            tc.tile_pool(name="otp", bufs=2) as ot_pool,
            tc.tile_pool(name="outp", bufs=2) as out_pool,
            tc.tile_pool(name="psmm", bufs=2, space="PSUM") as ps_pool,
            tc.tile_pool(name="psst", bufs=3, space="PSUM") as st_pool,
            tc.tile_pool(name="psor", bufs=3, space="PSUM") as or_pool,
        ):
            ones_t = const.tile([P, P], BF16)
            nc.vector.memset(ones_t[:], 1.0)

            kvres_cm = tc.tile_pool(name="kvres", bufs=1)
            kvres = kvres_cm.__enter__()
            kT = kvres.tile([P, N_KV, S], BF16)              # [d, kv, t]
            vN = kvres.tile([P, TT, N_KV * HEAD_DIM], BF16)  # [s, s_tile, kv*d]

            xres_cm = tc.tile_pool(name="xres", bufs=1)
            xres = xres_cm.__enter__()
            xt = xres.tile([P, KC, S], BF16)

            # ---- weight streaming helpers --------------------------------
            def load_head_slab(w_dram, idx):
                """[P, KC, 128] lhsT slab for one q/k head (2 DMAs)."""
                sl = wpool.tile([P, KC, HEAD_DIM], BF16, tag="wst", bufs=2)
                src = w_dram.ap()[idx] if isinstance(idx, int) \
                    else w_dram.ap()[idx[0], idx[1]]
                nc.sync.dma_start(out=sl[:, 0:KH, :], in_=src[:, 0:KH, :])
                nc.sync.dma_start(out=sl[:, KH:KC, :], in_=src[:, KH:KC, :])
                return sl

            def load_eighths(w_dram, cc):
                """8 x [P, 4, 512] moving-operand slices of one 512-col
                chunk (one dma_start each)."""
                out = []
                for e in range(8):
                    sl = wpool.tile([P, 4, NCH], BF16, tag="w8", bufs=9)
                    nc.sync.dma_start(
                        out=sl[:],
                        in_=w_dram.ap()[cc, e // 4, :,
                                        (e % 4) * 4:(e % 4) * 4 + 4, :],
                    )
                    out.append(sl)
                return out

            # ---- startup DMAs --------------------------------------------
            nc.sync.dma_start(out=xt[:, 0:KH, 0:P], in_=xt_d.ap()[:, 0:KH, 0:P])
            nc.sync.dma_start(out=xt[:, KH:KC, 0:P], in_=xt_d.ap()[:, KH:KC, 0:P])
            wv_e = [load_eighths(wv_d, 0)]
            for ts in (1, 2, 3):
                nc.sync.dma_start(out=xt[:, :, ts * P:(ts + 1) * P],
                                  in_=xt_d.ap()[:, :, ts * P:(ts + 1) * P])
            wv_e.append(load_eighths(wv_d, 1))
            for ts in (4, 5, 6, 7):
                nc.sync.dma_start(out=xt[:, :, ts * P:(ts + 1) * P],
                                  in_=xt_d.ap()[:, :, ts * P:(ts + 1) * P])
            c2 = const.tile([P, S], F32)
            nc.sync.dma_start(out=c2[:], in_=c2_d.ap())
            s2n = const.tile([P, S], F32)
            nc.sync.dma_start(out=s2n[:], in_=s2n_d.ap())

            # ---- compute helpers -----------------------------------------
            def proj_T_mms(slab, tc_, ps, k0, k1):
                """Transposed projection: out[d, t] += slab[kc].T @ xt."""
                for kc in range(k0, k1):
                    nc.tensor.matmul(
                        ps[:],
                        lhsT=slab[:, kc, :],
                        rhs=xt[:, kc, tc_ * NCH:(tc_ + 1) * NCH],
                        start=(kc == 0),
                        stop=(kc == KC - 1),
                    )

            def rope_t(ps, tc_, dest):
                """RoPE in [d, t] layout: dest = ps*c2 + swap_halves(ps)*s2n."""
                sw = scr_pool.tile([P, NCH], F32, tag="scr")
                nc.vector.tensor_copy(out=sw[0:64, :], in_=ps[64:P, :])
                nc.vector.tensor_copy(out=sw[64:P, :], in_=ps[0:64, :])
                m1 = scr_pool.tile([P, NCH], F32, tag="scr")
                nc.vector.tensor_tensor(m1[:], ps[:],
                                        c2[:, tc_ * NCH:(tc_ + 1) * NCH],
                                        mybir.AluOpType.mult)
                m2 = scr_pool.tile([P, NCH], F32, tag="scr")
                nc.vector.tensor_tensor(m2[:], sw[:],
                                        s2n[:, tc_ * NCH:(tc_ + 1) * NCH],
                                        mybir.AluOpType.mult)
                nc.vector.tensor_tensor(dest, m1[:], m2[:],
                                        mybir.AluOpType.add)

            def make_proj_filler(slab, tc_, dest):
                """Closures: 8x(4 proj matmuls) + rope. First closure
                allocates the psum tile."""
                hold = {}
                clos = []
                for j in range(8):
                    def mm(j=j):
                        if j == 0:
                            hold["ps"] = ps_pool.tile([P, NCH], F32, tag="mm",
                                                      name="pjps")
                        proj_T_mms(slab, tc_, hold["ps"], j * 4, (j + 1) * 4)
                    clos.append(mm)
                def rope():
                    rope_t(hold["ps"], tc_, dest)
                clos.append(rope)
                return clos

            def emit_unit(g, r, qc, qT_use, filler, direct_dest=None):
                """One attention unit (kv-group g, q-head r, q-col chunk qc)
                with PE filler closures woven between score matmuls."""
                pt = pt_pool.tile([P, TT, NCH], BF16, tag="pt")

                def score(st):
                    sps = st_pool.tile([P, NCH], F32, tag="st")
                    nc.tensor.matmul(
                        sps[:],
                        lhsT=kT[:, g, st * P:(st + 1) * P],
                        rhs=qT_use[:, r, qc * NCH:(qc + 1) * NCH],
                        start=True, stop=True,
                    )
                    nc.scalar.activation(
                        pt[:, st, :], sps[:],
                        mybir.ActivationFunctionType.Exp, scale=SCALE,
                    )

                fi = 0
                score(0)
                score(1)
                for st in range(2, TT):
                    if fi < len(filler):
                        filler[fi]()
                        fi += 1
                    score(st)
                while fi < len(filler):
                    filler[fi]()
                    fi += 1

                ops = or_pool.tile([P, NCH], F32, tag="or")
                rps = or_pool.tile([P, NCH], F32, tag="or")
                for st in range(TT):
                    nc.tensor.matmul(
                        ops[:], lhsT=vN[:, st, g * P:(g + 1) * P],
                        rhs=pt[:, st, :], start=(st == 0), stop=(st == TT - 1),
                    )
                    nc.tensor.matmul(
                        rps[:], lhsT=ones_t[:],
                        rhs=pt[:, st, :], start=(st == 0), stop=(st == TT - 1),
                    )
                rcb = rcb_pool.tile([P, NCH], F32, tag="rcb")
                nc.vector.reciprocal(out=rcb[:], in_=rps[:])
                if direct_dest is not None:
                    nc.vector.tensor_tensor(direct_dest, ops[:], rcb[:],
                                            mybir.AluOpType.mult)
                else:
                    ot = ot_pool.tile([P, NCH], BF16, tag="ot")
                    nc.vector.tensor_tensor(ot[:], ops[:], rcb[:],
                                            mybir.AluOpType.mult)
                    nc.sync.dma_start(out=otb_d.ap()[qc, :, g * REP + r, :],
                                      in_=ot[:])

            # ---- phase 1: v projection (natural layout) ------------------
            for cc in range(2):
                eighths = wv_e[cc]
                for tt in range(TT):
                    ps = ps_pool.tile([P, NCH], F32, tag="mm")
                    for kc in range(KC):
                        nc.tensor.matmul(
                            ps[:],
                            lhsT=xt[:, kc, tt * P:(tt + 1) * P],
                            rhs=eighths[kc // 4][:, kc % 4, :],
                            start=(kc == 0), stop=(kc == KC - 1),
                        )
                    nc.vector.tensor_copy(
                        out=vN[:, tt, cc * NCH:(cc + 1) * NCH], in_=ps[:])

            # queue q/k head slabs in consumption order (ring-gated)
            wk_slab = [load_head_slab(wk_d, h) for h in range(N_KV)]
            wq_slab = {}
            for h in range(REP):
                wq_slab[(0, h)] = load_head_slab(wq_d, (0, h))
            for h in range(REP):
                wq_slab[(1, h)] = load_head_slab(wq_d, (1, h))

            # ---- phase 2: k projection (transposed) + RoPE ---------------
            for h in range(N_KV):
                for tc_ in range(TC):
                    ps = ps_pool.tile([P, NCH], F32, tag="mm")
                    proj_T_mms(wk_slab[h], tc_, ps, 0, KC)
                    rope_t(ps, tc_, kT[:, h, tc_ * NCH:(tc_ + 1) * NCH])

            # ---- phase 3: q projection for group 0 -----------------------
            qT_cur = qtg_pool.tile([P, REP, S], BF16, tag="qtg")
            for h in range(REP):
                for tc_ in range(TC):
                    ps = ps_pool.tile([P, NCH], F32, tag="mm")
                    proj_T_mms(wq_slab[(0, h)], tc_, ps, 0, KC)
                    rope_t(ps, tc_, qT_cur[:, h, tc_ * NCH:(tc_ + 1) * NCH])

            # ---- groups 0..6: attention woven with next q projection -----
            for g in range(N_KV - 1):
                qT_next = qtg_pool.tile([P, REP, S], BF16, tag="qtg")
                fillers = [
                    make_proj_filler(
                        wq_slab[(g + 1, i // 2)], i % 2,
                        qT_next[:, i // 2, (i % 2) * NCH:(i % 2 + 1) * NCH])
                    for i in range(8)
                ]
                for i in range(8):
                    if g + 2 <= N_KV - 1 and i in (2, 4, 6, 7):
                        hh = {2: 0, 4: 1, 6: 2, 7: 3}[i]
                        wq_slab[(g + 2, hh)] = load_head_slab(wq_d, (g + 2, hh))
                    emit_unit(g, i // 2, i % 2, qT_cur, fillers[i])
                qT_cur = qT_next

            # ---- group 7 + wo --------------------------------------------
            xres_cm.__exit__(None, None, None)
            ores_cm = tc.tile_pool(name="ores", bufs=1)
            ores = ores_cm.__enter__()
            ot_all = ores.tile([P, N_HEADS, S], BF16)  # [d, h, t]

            def bounce(qc):
                for g in range(7):
                    nc.sync.dma_start(
                        out=ot_all[:, g * REP:(g + 1) * REP,
                                   qc * NCH:(qc + 1) * NCH],
                        in_=otb_d.ap()[qc, :, g * REP:(g + 1) * REP, :],
                    )

            wo_e = {}

            def make_wo_chain(nc5, tt_):
                """Closures: 8x(4 wo matmuls over heads) + copy/DMA tail."""
                eighths = wo_e[nc5]
                hold = {}
                clos = []
                for j in range(8):
                    def mm(j=j):
                        if j == 0:
                            hold["ps"] = ps_pool.tile([P, NCH], F32, tag="mm",
                                                      name="wops")
                        for h in range(j * 4, (j + 1) * 4):
                            nc.tensor.matmul(
                                hold["ps"][:],
                                lhsT=ot_all[:, h, tt_ * P:(tt_ + 1) * P],
                                rhs=eighths[h // 4][:, h % 4, :],
                                start=(h == 0), stop=(h == N_HEADS - 1),
                            )
                    clos.append(mm)
                def tail():
                    outt = out_pool.tile([P, NCH], F32, tag="outp")
                    nc.vector.tensor_copy(out=outt[:], in_=hold["ps"][:])
                    nc.sync.dma_start(
                        out=out_d.ap()[tt_ * P:(tt_ + 1) * P,
                                       nc5 * NCH:(nc5 + 1) * NCH],
                        in_=outt[:])
                clos.append(tail)
                return clos

            g7dest = lambda r, qc: ot_all[:, 28 + r, qc * NCH:(qc + 1) * NCH]

            bounce(0)
            wo_e[0] = load_eighths(wo_d, 0)
            ch = {tt_: make_wo_chain(0, tt_) for tt_ in range(TT)}
            emit_unit(7, 0, 0, qT_cur, [], direct_dest=g7dest(0, 0))
            emit_unit(7, 1, 0, qT_cur, [], direct_dest=g7dest(1, 0))
            emit_unit(7, 2, 0, qT_cur, ch[0][0:7], direct_dest=g7dest(2, 0))
            emit_unit(7, 3, 0, qT_cur, ch[1][0:7], direct_dest=g7dest(3, 0))
            for cl in ch[0][7:] + ch[1][7:]:
                cl()
            bounce(1)
            emit_unit(7, 0, 1, qT_cur, ch[2], direct_dest=g7dest(0, 1))
            emit_unit(7, 1, 1, qT_cur, ch[3], direct_dest=g7dest(1, 1))
            emit_unit(7, 2, 1, qT_cur, ch[4][0:7], direct_dest=g7dest(2, 1))
            emit_unit(7, 3, 1, qT_cur, ch[5][0:7], direct_dest=g7dest(3, 1))
            for cl in ch[4][7:] + ch[5][7:]:
                cl()
            for cl in ch[6] + ch[7]:
                cl()
            for nc5 in range(1, 8):
                wo_e[nc5] = load_eighths(wo_d, nc5)
                for tt_ in range(TT):
                    for cl in make_wo_chain(nc5, tt_):
                        cl()
            ores_cm.__exit__(None, None, None)
            kvres_cm.__exit__(None, None, None)

    nc.compile()
    return nc


# host-side input preparation -------------------------------------------------

_ROPE_PERM = np.concatenate([np.arange(0, HEAD_DIM, 2), np.arange(1, HEAD_DIM, 2)])


def _permute_heads(w, n_heads):
    """Permute columns within each head so rotation pairs become
    contiguous (even | odd) halves."""
    w = w.reshape(w.shape[0], n_heads, HEAD_DIM)
    return w[:, :, _ROPE_PERM].reshape(w.shape[0], n_heads * HEAD_DIM)


def _w_layout(w):
    """[DIM, C] f32 -> [C/512, 2, 128, 16, 512] bf16 moving-slab layout."""
    C = w.shape[1]
    wl = w.reshape(2, KH, P, C // NCH, NCH).transpose(3, 0, 2, 1, 4)
    return np.ascontiguousarray(wl).astype(ml_dtypes.bfloat16)


def _prep_shared(cos, sin, wq, wk, wv, wo):
    wq_p = _permute_heads(np.asarray(wq, dtype=np.float32), N_HEADS)
    wk_p = _permute_heads(np.asarray(wk, dtype=np.float32), N_KV)
    # lhsT chunk layouts: [g, h, k-part, kc, d] / [h, k-part, kc, d]
    wq_l = np.ascontiguousarray(
        wq_p.reshape(KC, P, N_KV, REP, HEAD_DIM).transpose(2, 3, 1, 0, 4)
    ).astype(ml_dtypes.bfloat16)
    wk_l = np.ascontiguousarray(
        wk_p.reshape(KC, P, N_KV, HEAD_DIM).transpose(2, 1, 0, 3)
    ).astype(ml_dtypes.bfloat16)
    wv_l = _w_layout(np.asarray(wv, dtype=np.float32))
    wo_l = _w_layout(np.asarray(wo, dtype=np.float32))
    # positions restart at 0 per block, so block 0's tables serve all cores
    c64 = np.asarray(cos[:S], dtype=np.float32).T          # [64, S]
    s64 = np.asarray(sin[:S], dtype=np.float32).T
    c2_l = np.ascontiguousarray(np.concatenate([c64, c64], axis=0))
    s2n_l = np.ascontiguousarray(np.concatenate([-s64, s64], axis=0))
    return c2_l, s2n_l, wq_l, wk_l, wv_l, wo_l


def _prep_x_block(xb):
    """x block [S, DIM] f32 -> xt [128, KC, S] bf16 (transposed)."""
    xtb = xb.T.reshape(KC, P, S).transpose(1, 0, 2)
    return np.ascontiguousarray(xtb).astype(ml_dtypes.bfloat16)


def kernel(x, cos, sin, wq, wk, wv, wo):
    if "nc" not in _CACHE:
        _CACHE["nc"] = build_nc()
    nc = _CACHE["nc"]

    x = np.asarray(x, dtype=np.float32)
    c2_l, s2n_l, wq_l, wk_l, wv_l, wo_l = _prep_shared(cos, sin, wq, wk, wv, wo)

    in_maps = []
    for b in range(B):
        in_maps.append({
            "xt": _prep_x_block(x[b * S:(b + 1) * S]),
            "c2": c2_l,
            "s2n": s2n_l,
            "wq": wq_l,
            "wk": wk_l,
            "wv": wv_l,
            "wo": wo_l,
        })
    _CACHE["last_in_maps"] = in_maps
    res = run_bass_kernel_spmd(nc, in_maps, core_ids=list(range(B)))
    _CACHE["last_results"] = res
    out = np.concatenate([res.results[b]["out"] for b in range(B)], axis=0)
    return out.astype(np.float32)


# revision 19
# speedup vs baseline: 1.1302x; 1.0503x over previous
"""Trainium2 Bass kernel for nn_Attention_45999099740384.

GQA attention over 8 independent packed sequences (block-diagonal mask with
equal blocks). Sharding: data-parallel over the 8 blocks - one block of
S=1024 tokens per NeuronCore, weights replicated, zero collectives.

Per-core pipeline (bf16 TensorEngine, fp32 PSUM):
  1. v projection in natural layout (xt stationary, wv moving).
  2. q/k projections TRANSPOSED (weight chunks stationary, xt moving) so
     q^T/k^T come out in [head_dim, t] layout directly - no PE transposes.
  3. RoPE applied in transposed layout on VectorE: host-permuted weight
     columns put rotation pairs into (even|odd) partition halves; the
     cross-partition half-swap is done with two 64-partition copies, then
     two mults against duplicated cos / sign-flipped sin tables and an add.
  4. scores computed transposed: ST[s,q] = kT.T @ qT -> ScalarE exp ->
     P^T tiles in SBUF; P@V needs no transpose of P.
  5. softmax row-sums via a ones[128,128] stationary matmul; reciprocal on
     VectorE (keeps ScalarE exp-only: a single activation table, no
     ACT_TABLE_LOAD churn); normalization deferred to after P@V.
  6. wo matmul from the transposed attention output.

Scheduling: the q projection of group g+1 is woven between the score
matmuls of group g's attention units so the PE never waits for ScalarE
exp; for the last group the leading wo matmul chains are woven in instead.
Attention outputs of groups 0-6 bounce through DRAM (SBUF cannot hold
ot_all while xt is still resident); group 7 writes ot_all directly.
"""

import numpy as np
import ml_dtypes

import concourse.bass as bass
import concourse.mybir as mybir
import concourse.tile as tile
from concourse import bacc
from concourse.bass_utils import run_bass_kernel_spmd

# problem constants (hardcoded per task instructions)
DIM = 4096
N_HEADS = 32
HEAD_DIM = 128
N_KV = 8
REP = 4
B = 8
S = 1024
T = B * S

P = 128                  # SBUF partitions
KC = DIM // P            # 32 contraction chunks of 128
KH = KC // 2             # 16 (w_layout half-chunk count)
TT = S // P              # 8 token tiles of 128
TC = S // 512            # 2 token chunks of 512
NCH = 512                # matmul moving free dim
SCALE = HEAD_DIM ** -0.5

F32 = mybir.dt.float32
BF16 = mybir.dt.bfloat16

_CACHE = {}


def build_nc():
    nc = bacc.Bacc("TRN2", target_bir_lowering=False, debug=False, num_devices=8)

    xt_d = nc.dram_tensor("xt", [P, KC, S], BF16, kind="ExternalInput")
    c2_d = nc.dram_tensor("c2", [P, S], F32, kind="ExternalInput")
    s2n_d = nc.dram_tensor("s2n", [P, S], F32, kind="ExternalInput")
    # wq: [g, h, k-part, kc, d] lhsT chunks (rope-permuted d columns)
    wq_d = nc.dram_tensor("wq", [N_KV, REP, P, KC, HEAD_DIM], BF16,
                          kind="ExternalInput")
    wk_d = nc.dram_tensor("wk", [N_KV, P, KC, HEAD_DIM], BF16,
                          kind="ExternalInput")
    # wv/wo: moving-operand slabs [chunk, half, 128, 16, 512]
    wv_d = nc.dram_tensor("wv", [2, 2, P, KH, NCH], BF16, kind="ExternalInput")
    # wo: [ct, d, h, c] lhsT slabs (stationary, streamed per 128-col tile)
    wo_d = nc.dram_tensor("wo", [KC, P, N_HEADS, P], BF16, kind="ExternalInput")
    # transposed output [DIM, S]; host untransposes
    out_d = nc.dram_tensor("out", [DIM, S], F32, kind="ExternalOutput")
    # attention-output bounce for groups 0..6
    otb_d = nc.dram_tensor("otb", [2, P, 28, NCH], BF16)

    with tile.TileContext(nc) as tc:
        with (
            tc.tile_pool(name="const", bufs=1) as const,
            tc.tile_pool(name="wpool", bufs=1) as wpool,
            tc.tile_pool(name="qtg", bufs=2) as qtg_pool,
            tc.tile_pool(name="ptp", bufs=2) as pt_pool,
            tc.tile_pool(name="scr", bufs=3) as scr_pool,
            tc.tile_pool(name="rcbp", bufs=1) as rcb_pool,
            tc.tile_pool(name="otp", bufs=1) as ot_pool,
            tc.tile_pool(name="outp", bufs=1) as out_pool,
            tc.tile_pool(name="pt8p", bufs=1) as pt8_pool,
            tc.tile_pool(name="psmm", bufs=2, space="PSUM") as ps_pool,
            tc.tile_pool(name="psst", bufs=3, space="PSUM") as st_pool,
            tc.tile_pool(name="psor", bufs=3, space="PSUM") as or_pool,
        ):
            ones8 = const.tile([P, 2, P], mybir.dt.float8e5)
            nc.vector.memset(ones8[:], 1.0)
            # exp bias -ln(128): keeps exp within fp8e4 range for the fp8
            # row-sum copy; cancels in the softmax normalization
            ebias = const.tile([P, 1], F32)
            nc.vector.memset(ebias[:], -4.852030263919617)

            kvres_cm = tc.tile_pool(name="kvres", bufs=1)
            kvres = kvres_cm.__enter__()
            kT = kvres.tile([P, N_KV, S], BF16)              # [d, kv, t]
            vN = kvres.tile([P, TT, N_KV * HEAD_DIM], BF16)  # [s, s_tile, kv*d]

            xres_cm = tc.tile_pool(name="xres", bufs=1)
            xres = xres_cm.__enter__()
            xt = xres.tile([P, KC, S], BF16)

            # ---- weight streaming helpers --------------------------------
            def load_head_slab(w_dram, idx):
                """[P, KC, 128] lhsT slab for one q/k head (2 DMAs)."""
                sl = wpool.tile([P, KC, HEAD_DIM], BF16, tag="wst", bufs=2)
                src = w_dram.ap()[idx] if isinstance(idx, int) \
                    else w_dram.ap()[idx[0], idx[1]]
                nc.sync.dma_start(out=sl[:, 0:KH, :], in_=src[:, 0:KH, :])
                nc.sync.dma_start(out=sl[:, KH:KC, :], in_=src[:, KH:KC, :])
                return sl

            # ---- startup DMAs (xt slices interleaved with wv slabs) ------
            def load_eighth(w_dram, cc, e):
                sl = wpool.tile([P, 4, NCH], BF16, tag="w8", bufs=9, name="w8t")
                nc.sync.dma_start(
                    out=sl[:],
                    in_=w_dram.ap()[cc, e // 4, :,
                                    (e % 4) * 4:(e % 4) * 4 + 4, :])
                return sl

            def xslice(ts):
                nc.sync.dma_start(out=xt[:, :, ts * P:(ts + 1) * P],
                                  in_=xt_d.ap()[:, :, ts * P:(ts + 1) * P])

            nc.sync.dma_start(out=xt[:, 0:KH, 0:P], in_=xt_d.ap()[:, 0:KH, 0:P])
            nc.sync.dma_start(out=xt[:, KH:KC, 0:P], in_=xt_d.ap()[:, KH:KC, 0:P])
            wv_e = [[], []]
            for e in range(4):
                wv_e[0].append(load_eighth(wv_d, 0, e))
            xslice(1)
            wv_e[0].append(load_eighth(wv_d, 0, 4))
            wv_e[0].append(load_eighth(wv_d, 0, 5))
            xslice(2)
            wv_e[0].append(load_eighth(wv_d, 0, 6))
            wv_e[0].append(load_eighth(wv_d, 0, 7))
            xslice(3)
            for e in range(4):
                wv_e[1].append(load_eighth(wv_d, 1, e))
            xslice(4)
            xslice(5)
            for e in range(4, 8):
                wv_e[1].append(load_eighth(wv_d, 1, e))
            xslice(6)
            xslice(7)
            c2 = const.tile([P, S], F32)
            nc.sync.dma_start(out=c2[:], in_=c2_d.ap())
            s2n = const.tile([P, S], F32)
            nc.sync.dma_start(out=s2n[:], in_=s2n_d.ap())

            # ---- compute helpers -----------------------------------------
            def proj_T_mms(slab, tc_, ps, k0, k1):
                """Transposed projection: out[d, t] += slab[kc].T @ xt."""
                for kc in range(k0, k1):
                    nc.tensor.matmul(
                        ps[:],
                        lhsT=slab[:, kc, :],
                        rhs=xt[:, kc, tc_ * NCH:(tc_ + 1) * NCH],
                        start=(kc == 0),
                        stop=(kc == KC - 1),
                    )

            def rope_t(ps, tc_, dest):
                """RoPE in [d, t] layout: dest = ps*c2 + swap_halves(ps)*s2n."""
                sw = scr_pool.tile([P, NCH], F32, tag="scr")
                nc.vector.tensor_copy(out=sw[0:64, :], in_=ps[64:P, :])
                nc.vector.tensor_copy(out=sw[64:P, :], in_=ps[0:64, :])
                m1 = scr_pool.tile([P, NCH], F32, tag="scr")
                nc.vector.tensor_tensor(m1[:], ps[:],
                                        c2[:, tc_ * NCH:(tc_ + 1) * NCH],
                                        mybir.AluOpType.mult)
                m2 = scr_pool.tile([P, NCH], F32, tag="scr")
                nc.vector.tensor_tensor(m2[:], sw[:],
                                        s2n[:, tc_ * NCH:(tc_ + 1) * NCH],
                                        mybir.AluOpType.mult)
                nc.vector.tensor_tensor(dest, m1[:], m2[:],
                                        mybir.AluOpType.add)

            def make_proj_filler(slab, tc_, dest):
                """Closures: 8x(4 proj matmuls) + rope. First closure
                allocates the psum tile."""
                hold = {}
                clos = []
                for j in range(8):
                    def mm(j=j):
                        if j == 0:
                            hold["ps"] = ps_pool.tile([P, NCH], F32, tag="mm",
                                                      name="pjps")
                        proj_T_mms(slab, tc_, hold["ps"], j * 4, (j + 1) * 4)
                    clos.append(mm)
                def rope():
                    rope_t(hold["ps"], tc_, dest)
                clos.append(rope)
                return clos

            def emit_unit(g, r, qc, qT_use, filler, direct_dest=None):
                """One attention unit (kv-group g, q-head r, q-col chunk qc)
                with PE filler closures woven between score matmuls."""
                pt = pt_pool.tile([P, TT, NCH], BF16, tag="pt")

                def score(st):
                    sps = st_pool.tile([P, NCH], F32, tag="st")
                    nc.tensor.matmul(
                        sps[:],
                        lhsT=kT[:, g, st * P:(st + 1) * P],
                        rhs=qT_use[:, r, qc * NCH:(qc + 1) * NCH],
                        start=True, stop=True,
                    )
                    nc.scalar.activation(
                        pt[:, st, :], sps[:],
                        mybir.ActivationFunctionType.Exp, scale=SCALE,
                        bias=ebias[:],
                    )

                fi = 0
                score(0)
                score(1)
                for st in range(2, TT):
                    if fi < len(filler):
                        filler[fi]()
                        fi += 1
                    score(st)
                # fp8 copy of P^T for the double-row row-sum matmuls
                pt8 = pt8_pool.tile([P, TT, NCH], mybir.dt.float8e5, tag="pt8")
                nc.vector.tensor_copy(out=pt8[:], in_=pt[:])
                while fi < len(filler):
                    filler[fi]()
                    fi += 1

                ops = or_pool.tile([P, NCH], F32, tag="or")
                rps = or_pool.tile([P, NCH], F32, tag="or")
                for st in range(TT):
                    nc.tensor.matmul(
                        ops[:], lhsT=vN[:, st, g * P:(g + 1) * P],
                        rhs=pt[:, st, :], start=(st == 0), stop=(st == TT - 1),
                    )
                for j in range(TT // 2):
                    nc.tensor.matmul(
                        rps[:], lhsT=ones8[:],
                        rhs=pt8[:, 2 * j:2 * j + 2, :],
                        start=(j == 0), stop=(j == TT // 2 - 1),
                        perf_mode=mybir.MatmulPerfMode.DoubleRow,
                    )
                rcb = rcb_pool.tile([P, NCH], F32, tag="rcb")
                nc.vector.reciprocal_approx_fast(out=rcb[:], in_=rps[:])
                if direct_dest is not None:
                    nc.vector.tensor_tensor(direct_dest, ops[:], rcb[:],
                                            mybir.AluOpType.mult)
                else:
                    ot = ot_pool.tile([P, NCH], BF16, tag="ot")
                    nc.vector.tensor_tensor(ot[:], ops[:], rcb[:],
                                            mybir.AluOpType.mult)
                    nc.sync.dma_start(out=otb_d.ap()[qc, :, g * REP + r, :],
                                      in_=ot[:])

            # ---- phase 1: v projection (natural layout) ------------------
            for cc in range(2):
                eighths = wv_e[cc]
                for tt in range(TT):
                    ps = ps_pool.tile([P, NCH], F32, tag="mm")
                    for kc in range(KC):
                        nc.tensor.matmul(
                            ps[:],
                            lhsT=xt[:, kc, tt * P:(tt + 1) * P],
                            rhs=eighths[kc // 4][:, kc % 4, :],
                            start=(kc == 0), stop=(kc == KC - 1),
                        )
                    nc.vector.tensor_copy(
                        out=vN[:, tt, cc * NCH:(cc + 1) * NCH], in_=ps[:])

            # queue q/k head slabs in consumption order (ring-gated)
            wk_slab = [load_head_slab(wk_d, h) for h in range(N_KV)]
            wq_slab = {}
            for h in range(REP):
                wq_slab[(0, h)] = load_head_slab(wq_d, (0, h))
            for h in range(REP):
                wq_slab[(1, h)] = load_head_slab(wq_d, (1, h))

            # ---- phase 2: k projection (transposed) + RoPE ---------------
            for h in range(N_KV):
                for tc_ in range(TC):
                    ps = ps_pool.tile([P, NCH], F32, tag="mm")
                    proj_T_mms(wk_slab[h], tc_, ps, 0, KC)
                    rope_t(ps, tc_, kT[:, h, tc_ * NCH:(tc_ + 1) * NCH])

            # ---- phase 3: q projection for group 0 -----------------------
            qT_cur = qtg_pool.tile([P, REP, S], BF16, tag="qtg")
            for h in range(REP):
                for tc_ in range(TC):
                    ps = ps_pool.tile([P, NCH], F32, tag="mm")
                    proj_T_mms(wq_slab[(0, h)], tc_, ps, 0, KC)
                    rope_t(ps, tc_, qT_cur[:, h, tc_ * NCH:(tc_ + 1) * NCH])

            # ---- groups 0..6: attention woven with next q projection -----
            for g in range(N_KV - 1):
                qT_next = qtg_pool.tile([P, REP, S], BF16, tag="qtg")
                fillers = [
                    make_proj_filler(
                        wq_slab[(g + 1, i // 2)], i % 2,
                        qT_next[:, i // 2, (i % 2) * NCH:(i % 2 + 1) * NCH])
                    for i in range(8)
                ]
                for i in range(8):
                    if g + 2 <= N_KV - 1 and i in (2, 4, 6, 7):
                        hh = {2: 0, 4: 1, 6: 2, 7: 3}[i]
                        wq_slab[(g + 2, hh)] = load_head_slab(wq_d, (g + 2, hh))
                    emit_unit(g, i // 2, i % 2, qT_cur, fillers[i])
                qT_cur = qT_next

            # ---- group 7 + wo --------------------------------------------
            xres_cm.__exit__(None, None, None)
            ores_cm = tc.tile_pool(name="ores", bufs=1)
            ores = ores_cm.__enter__()
            ot_all = ores.tile([P, N_HEADS, S], BF16)  # [d, h, t]

            def bounce(qc):
                for g in range(7):
                    nc.sync.dma_start(
                        out=ot_all[:, g * REP:(g + 1) * REP,
                                   qc * NCH:(qc + 1) * NCH],
                        in_=otb_d.ap()[qc, :, g * REP:(g + 1) * REP, :],
                    )

            def make_wo_chain(slab, ct, tc_):
                """Closures: 8x(4 wo matmuls over heads, slab stationary)
                + copy/DMA tail writing the transposed output."""
                hold = {}
                clos = []
                for j in range(8):
                    def mm(j=j):
                        if j == 0:
                            hold["ps"] = ps_pool.tile([P, NCH], F32, tag="mm",
                                                      name="wops")
                        for h in range(j * 4, (j + 1) * 4):
                            nc.tensor.matmul(
                                hold["ps"][:],
                                lhsT=slab[:, h, :],
                                rhs=ot_all[:, h, tc_ * NCH:(tc_ + 1) * NCH],
                                start=(h == 0), stop=(h == N_HEADS - 1),
                            )
                    clos.append(mm)
                def tail():
                    outt = out_pool.tile([P, NCH], F32, tag="outp")
                    nc.vector.tensor_copy(out=outt[:], in_=hold["ps"][:])
                    nc.sync.dma_start(
                        out=out_d.ap()[ct * P:(ct + 1) * P,
                                       tc_ * NCH:(tc_ + 1) * NCH],
                        in_=outt[:])
                clos.append(tail)
                return clos

            g7dest = lambda r, qc: ot_all[:, 28 + r, qc * NCH:(qc + 1) * NCH]

            bounce(0)
            wo_slab0 = load_head_slab(wo_d, 0)
            wo_slab1 = load_head_slab(wo_d, 1)
            ch00 = make_wo_chain(wo_slab0, 0, 0)
            ch10 = make_wo_chain(wo_slab1, 1, 0)
            emit_unit(7, 0, 0, qT_cur, [], direct_dest=g7dest(0, 0))
            emit_unit(7, 1, 0, qT_cur, [], direct_dest=g7dest(1, 0))
            emit_unit(7, 2, 0, qT_cur, ch00[0:7], direct_dest=g7dest(2, 0))
            emit_unit(7, 3, 0, qT_cur, ch10[0:7], direct_dest=g7dest(3, 0))
            for cl in ch00[7:] + ch10[7:]:
                cl()
            bounce(1)
            ch01 = make_wo_chain(wo_slab0, 0, 1)
            ch11 = make_wo_chain(wo_slab1, 1, 1)
            emit_unit(7, 0, 1, qT_cur, ch01[0:7], direct_dest=g7dest(0, 1))
            emit_unit(7, 1, 1, qT_cur, ch11[0:7], direct_dest=g7dest(1, 1))
            emit_unit(7, 2, 1, qT_cur, [], direct_dest=g7dest(2, 1))
            emit_unit(7, 3, 1, qT_cur, [], direct_dest=g7dest(3, 1))
            for cl in ch01[7:] + ch11[7:]:
                cl()
            for ct in range(2, KC):
                slab = load_head_slab(wo_d, ct)
                for tc_ in range(TC):
                    for cl in make_wo_chain(slab, ct, tc_):
                        cl()
            ores_cm.__exit__(None, None, None)
            kvres_cm.__exit__(None, None, None)

    nc.compile()
    return nc


# host-side input preparation -------------------------------------------------

_ROPE_PERM = np.concatenate([np.arange(0, HEAD_DIM, 2), np.arange(1, HEAD_DIM, 2)])


def _permute_heads(w, n_heads):
    """Permute columns within each head so rotation pairs become
    contiguous (even | odd) halves."""
    w = w.reshape(w.shape[0], n_heads, HEAD_DIM)
    return w[:, :, _ROPE_PERM].reshape(w.shape[0], n_heads * HEAD_DIM)


def _w_layout(w):
    """[DIM, C] f32 -> [C/512, 2, 128, 16, 512] bf16 moving-slab layout."""
    C = w.shape[1]
    wl = w.reshape(2, KH, P, C // NCH, NCH).transpose(3, 0, 2, 1, 4)
    return np.ascontiguousarray(wl).astype(ml_dtypes.bfloat16)


def _prep_shared(cos, sin, wq, wk, wv, wo):
    wq_p = _permute_heads(np.asarray(wq, dtype=np.float32), N_HEADS)
    wk_p = _permute_heads(np.asarray(wk, dtype=np.float32), N_KV)
    # lhsT chunk layouts: [g, h, k-part, kc, d] / [h, k-part, kc, d]
    wq_l = np.ascontiguousarray(
        wq_p.reshape(KC, P, N_KV, REP, HEAD_DIM).transpose(2, 3, 1, 0, 4)
    ).astype(ml_dtypes.bfloat16)
    wk_l = np.ascontiguousarray(
        wk_p.reshape(KC, P, N_KV, HEAD_DIM).transpose(2, 1, 0, 3)
    ).astype(ml_dtypes.bfloat16)
    wv_l = _w_layout(np.asarray(wv, dtype=np.float32))
    # wo lhsT slabs [ct, d, h, c]
    wo_l = np.ascontiguousarray(
        np.asarray(wo, dtype=np.float32)
        .reshape(N_HEADS, HEAD_DIM, KC, P).transpose(2, 1, 0, 3)
    ).astype(ml_dtypes.bfloat16)
    # positions restart at 0 per block, so block 0's tables serve all cores
    c64 = np.asarray(cos[:S], dtype=np.float32).T          # [64, S]
    s64 = np.asarray(sin[:S], dtype=np.float32).T
    c2_l = np.ascontiguousarray(np.concatenate([c64, c64], axis=0))
    s2n_l = np.ascontiguousarray(np.concatenate([-s64, s64], axis=0))
    return c2_l, s2n_l, wq_l, wk_l, wv_l, wo_l


def _prep_x_block(xb):
    """x block [S, DIM] f32 -> xt [128, KC, S] bf16 (transposed)."""
    xtb = xb.T.reshape(KC, P, S).transpose(1, 0, 2)
    return np.ascontiguousarray(xtb).astype(ml_dtypes.bfloat16)


def kernel(x, cos, sin, wq, wk, wv, wo):
    if "nc" not in _CACHE:
        _CACHE["nc"] = build_nc()
    nc = _CACHE["nc"]

    x = np.asarray(x, dtype=np.float32)
    c2_l, s2n_l, wq_l, wk_l, wv_l, wo_l = _prep_shared(cos, sin, wq, wk, wv, wo)

    in_maps = []
    for b in range(B):
        in_maps.append({
            "xt": _prep_x_block(x[b * S:(b + 1) * S]),
            "c2": c2_l,
            "s2n": s2n_l,
            "wq": wq_l,
            "wk": wk_l,
            "wv": wv_l,
            "wo": wo_l,
        })
    _CACHE["last_in_maps"] = in_maps
    res = run_bass_kernel_spmd(nc, in_maps, core_ids=list(range(B)))
    _CACHE["last_results"] = res
    # per-core output is transposed [DIM, S]
    out = np.concatenate([res.results[b]["out"].T for b in range(B)], axis=0)
    return np.ascontiguousarray(out, dtype=np.float32)


# revision 28
# speedup vs baseline: 1.1331x; 1.0026x over previous
"""Trainium2 Bass kernel for nn_Attention_45999099740384.

GQA attention over 8 independent packed sequences (block-diagonal mask with
equal blocks). Sharding: data-parallel over the 8 blocks - one block of
S=1024 tokens per NeuronCore, weights replicated, zero collectives.

Per-core pipeline (bf16 TensorEngine, fp32 PSUM):
  1. v projection in natural layout (xt stationary, wv moving).
  2. q/k projections TRANSPOSED (weight chunks stationary, xt moving) so
     q^T/k^T come out in [head_dim, t] layout directly - no PE transposes.
  3. RoPE applied in transposed layout on VectorE: host-permuted weight
     columns put rotation pairs into (even|odd) partition halves; the
     cross-partition half-swap is done with two 64-partition copies, then
     two mults against duplicated cos / sign-flipped sin tables and an add.
  4. scores computed transposed: ST[s,q] = kT.T @ qT -> ScalarE exp ->
     P^T tiles in SBUF; P@V needs no transpose of P.
  5. softmax row-sums via a ones[128,128] stationary matmul; reciprocal on
     VectorE (keeps ScalarE exp-only: a single activation table, no
     ACT_TABLE_LOAD churn); normalization deferred to after P@V.
  6. wo matmul from the transposed attention output.

Scheduling: the q projection of group g+1 is woven between the score
matmuls of group g's attention units so the PE never waits for ScalarE
exp; for the last group the leading wo matmul chains are woven in instead.
Attention outputs of groups 0-6 bounce through DRAM (SBUF cannot hold
ot_all while xt is still resident); group 7 writes ot_all directly.
"""

import numpy as np
import ml_dtypes

import concourse.bass as bass
import concourse.mybir as mybir
import concourse.tile as tile
from concourse import bacc
from concourse.bass_utils import run_bass_kernel_spmd

# problem constants (hardcoded per task instructions)
DIM = 4096
N_HEADS = 32
HEAD_DIM = 128
N_KV = 8
REP = 4
B = 8
S = 1024
T = B * S

P = 128                  # SBUF partitions
KC = DIM // P            # 32 contraction chunks of 128
KH = KC // 2             # 16 (w_layout half-chunk count)
TT = S // P              # 8 token tiles of 128
TC = S // 512            # 2 token chunks of 512
NCH = 512                # matmul moving free dim
SCALE = HEAD_DIM ** -0.5

F32 = mybir.dt.float32
BF16 = mybir.dt.bfloat16

_CACHE = {}


def build_nc():
    nc = bacc.Bacc("TRN2", target_bir_lowering=False, debug=False, num_devices=8)

    xt_d = nc.dram_tensor("xt", [P, KC, S], BF16, kind="ExternalInput")
    c2_d = nc.dram_tensor("c2", [P, S], BF16, kind="ExternalInput")
    s2n_d = nc.dram_tensor("s2n", [P, S], BF16, kind="ExternalInput")
    # wq: [g, h, k-part, kc, d] lhsT chunks (rope-permuted d columns)
    wq_d = nc.dram_tensor("wq", [N_KV, REP, P, KC, HEAD_DIM], BF16,
                          kind="ExternalInput")
    wk_d = nc.dram_tensor("wk", [N_KV, P, KC, HEAD_DIM], BF16,
                          kind="ExternalInput")
    # wv/wo: moving-operand slabs [chunk, half, 128, 16, 512]
    wv_d = nc.dram_tensor("wv", [2, 2, P, KH, NCH], BF16, kind="ExternalInput")
    # wo: [ct, d, h, c] lhsT slabs (stationary, streamed per 128-col tile)
    wo_d = nc.dram_tensor("wo", [KC, P, N_HEADS, P], BF16, kind="ExternalInput")
    # transposed output [DIM, S]; host untransposes
    out_d = nc.dram_tensor("out", [DIM, S], F32, kind="ExternalOutput")
    # attention-output bounce for groups 0..6
    otb_d = nc.dram_tensor("otb", [2, P, 28, NCH], BF16)

    with tile.TileContext(nc) as tc:
        with (
            tc.tile_pool(name="const", bufs=1) as const,
            tc.tile_pool(name="wpool", bufs=1) as wpool,
            tc.tile_pool(name="qtg", bufs=2) as qtg_pool,
            tc.tile_pool(name="ptp", bufs=2) as pt_pool,
            tc.tile_pool(name="scr", bufs=3) as scr_pool,
            tc.tile_pool(name="rcbp", bufs=1) as rcb_pool,
            tc.tile_pool(name="otp", bufs=1) as ot_pool,
            tc.tile_pool(name="outp", bufs=2) as out_pool,
            tc.tile_pool(name="pt8p", bufs=1) as pt8_pool,
            tc.tile_pool(name="psmm", bufs=2, space="PSUM") as ps_pool,
            tc.tile_pool(name="psst", bufs=3, space="PSUM") as st_pool,
            tc.tile_pool(name="psor", bufs=3, space="PSUM") as or_pool,
        ):
            ones8 = const.tile([P, 2, P], mybir.dt.float8e5)
            nc.vector.memset(ones8[:], 1.0)
            # exp bias -ln(128): keeps exp within fp8e4 range for the fp8
            # row-sum copy; cancels in the softmax normalization
            ebias = const.tile([P, 1], F32)
            nc.vector.memset(ebias[:], -4.852030263919617)

            kvres_cm = tc.tile_pool(name="kvres", bufs=1)
            kvres = kvres_cm.__enter__()
            kT = kvres.tile([P, N_KV, S], BF16)              # [d, kv, t]
            vN = kvres.tile([P, TT, N_KV * HEAD_DIM], BF16)  # [s, s_tile, kv*d]

            xres_cm = tc.tile_pool(name="xres", bufs=1)
            xres = xres_cm.__enter__()
            xt = xres.tile([P, KC, S], BF16)

            # ---- weight streaming helpers --------------------------------
            def load_head_slab(w_dram, idx):
                """[P, KC, 128] lhsT slab for one q/k head (2 DMAs)."""
                sl = wpool.tile([P, KC, HEAD_DIM], BF16, tag="wst", bufs=3)
                src = w_dram.ap()[idx] if isinstance(idx, int) \
                    else w_dram.ap()[idx[0], idx[1]]
                nc.sync.dma_start(out=sl[:, 0:KH, :], in_=src[:, 0:KH, :])
                nc.sync.dma_start(out=sl[:, KH:KC, :], in_=src[:, KH:KC, :])
                return sl

            # ---- startup DMAs (xt slices interleaved with wv slabs) ------
            def load_sixteenth(w_dram, cc, e):
                sl = wpool.tile([P, 2, NCH], BF16, tag="w16", bufs=17,
                                name="w16t")
                nc.sync.dma_start(
                    out=sl[:],
                    in_=w_dram.ap()[cc, e // 8, :,
                                    (e % 8) * 2:(e % 8) * 2 + 2, :])
                return sl

            def xslice(ts):
                nc.sync.dma_start(out=xt[:, :, ts * P:(ts + 1) * P],
                                  in_=xt_d.ap()[:, :, ts * P:(ts + 1) * P])

            nc.sync.dma_start(out=xt[:, 0:KH, 0:P], in_=xt_d.ap()[:, 0:KH, 0:P])
            nc.sync.dma_start(out=xt[:, KH:KC, 0:P], in_=xt_d.ap()[:, KH:KC, 0:P])
            wv_e = [[], []]
            for e in range(6):
                wv_e[0].append(load_sixteenth(wv_d, 0, e))
            xslice(1)
            for e in range(6, 11):
                wv_e[0].append(load_sixteenth(wv_d, 0, e))
            xslice(2)
            for e in range(11, 16):
                wv_e[0].append(load_sixteenth(wv_d, 0, e))
            xslice(3)
            for e in range(8):
                wv_e[1].append(load_sixteenth(wv_d, 1, e))
            xslice(4)
            xslice(5)
            for e in range(8, 16):
                wv_e[1].append(load_sixteenth(wv_d, 1, e))
            xslice(6)
            xslice(7)
            c2 = const.tile([P, S], BF16)
            nc.sync.dma_start(out=c2[:], in_=c2_d.ap())
            s2n = const.tile([P, S], BF16)
            nc.sync.dma_start(out=s2n[:], in_=s2n_d.ap())

            # ---- compute helpers -----------------------------------------
            def proj_T_mms(slab, tc_, ps, k0, k1):
                """Transposed projection: out[d, t] += slab[kc].T @ xt."""
                for kc in range(k0, k1):
                    nc.tensor.matmul(
                        ps[:],
                        lhsT=slab[:, kc, :],
                        rhs=xt[:, kc, tc_ * NCH:(tc_ + 1) * NCH],
                        start=(kc == 0),
                        stop=(kc == KC - 1),
                    )

            def rope_t(ps, tc_, dest):
                """RoPE in [d, t] layout: dest = ps*c2 + swap_halves(ps)*s2n."""
                sw = scr_pool.tile([P, NCH], F32, tag="scr")
                nc.vector.tensor_copy(out=sw[0:64, :], in_=ps[64:P, :])
                nc.vector.tensor_copy(out=sw[64:P, :], in_=ps[0:64, :])
                m1 = scr_pool.tile([P, NCH], F32, tag="scr")
                nc.vector.tensor_tensor(m1[:], ps[:],
                                        c2[:, tc_ * NCH:(tc_ + 1) * NCH],
                                        mybir.AluOpType.mult)
                m2 = scr_pool.tile([P, NCH], F32, tag="scr")
                nc.vector.tensor_tensor(m2[:], sw[:],
                                        s2n[:, tc_ * NCH:(tc_ + 1) * NCH],
                                        mybir.AluOpType.mult)
                nc.vector.tensor_tensor(dest, m1[:], m2[:],
                                        mybir.AluOpType.add)

            def make_proj_filler(slab, tc_, dest):
                """Closures: 8x(4 proj matmuls) + rope. First closure
                allocates the psum tile."""
                hold = {}
                clos = []
                for j in range(8):
                    def mm(j=j):
                        if j == 0:
                            hold["ps"] = ps_pool.tile([P, NCH], F32, tag="mm",
                                                      name="pjps")
                        proj_T_mms(slab, tc_, hold["ps"], j * 4, (j + 1) * 4)
                    clos.append(mm)
                def rope():
                    rope_t(hold["ps"], tc_, dest)
                clos.append(rope)
                return clos

            def emit_unit(g, r, qc, qT_use, filler, direct_dest=None):
                """One attention unit (kv-group g, q-head r, q-col chunk qc)
                with PE filler closures woven between score matmuls."""
                pt = pt_pool.tile([P, TT, NCH], BF16, tag="pt")

                def score(st):
                    sps = st_pool.tile([P, NCH], F32, tag="st")
                    nc.tensor.matmul(
                        sps[:],
                        lhsT=kT[:, g, st * P:(st + 1) * P],
                        rhs=qT_use[:, r, qc * NCH:(qc + 1) * NCH],
                        start=True, stop=True,
                    )
                    nc.scalar.activation(
                        pt[:, st, :], sps[:],
                        mybir.ActivationFunctionType.Exp, scale=SCALE,
                        bias=ebias[:],
                    )

                fi = 0
                score(0)
                score(1)
                for st in range(2, TT):
                    if fi < len(filler):
                        filler[fi]()
                        fi += 1
                    score(st)
                # fp8 copy of P^T for the double-row row-sum matmuls
                pt8 = pt8_pool.tile([P, TT, NCH], mybir.dt.float8e5, tag="pt8")
                nc.vector.tensor_copy(out=pt8[:], in_=pt[:])
                while fi < len(filler):
                    filler[fi]()
                    fi += 1

                ops = or_pool.tile([P, NCH], F32, tag="or")
                rps = or_pool.tile([P, NCH], F32, tag="or")
                for st in range(TT):
                    nc.tensor.matmul(
                        ops[:], lhsT=vN[:, st, g * P:(g + 1) * P],
                        rhs=pt[:, st, :], start=(st == 0), stop=(st == TT - 1),
                    )
                for j in range(TT // 2):
                    nc.tensor.matmul(
                        rps[:], lhsT=ones8[:],
                        rhs=pt8[:, 2 * j:2 * j + 2, :],
                        start=(j == 0), stop=(j == TT // 2 - 1),
                        perf_mode=mybir.MatmulPerfMode.DoubleRow,
                    )
                rcb = rcb_pool.tile([P, NCH], F32, tag="rcb")
                nc.vector.reciprocal_approx_fast(out=rcb[:], in_=rps[:])
                if direct_dest is not None:
                    nc.vector.tensor_tensor(direct_dest, ops[:], rcb[:],
                                            mybir.AluOpType.mult)
                else:
                    ot = ot_pool.tile([P, NCH], BF16, tag="ot")
                    nc.vector.tensor_tensor(ot[:], ops[:], rcb[:],
                                            mybir.AluOpType.mult)
                    nc.sync.dma_start(out=otb_d.ap()[qc, :, g * REP + r, :],
                                      in_=ot[:])

            # ---- phase 1: v projection (natural layout) ------------------
            for cc in range(2):
                sixteenths = wv_e[cc]
                for tt in range(TT):
                    ps = ps_pool.tile([P, NCH], F32, tag="mm")
                    for kc in range(KC):
                        nc.tensor.matmul(
                            ps[:],
                            lhsT=xt[:, kc, tt * P:(tt + 1) * P],
                            rhs=sixteenths[kc // 2][:, kc % 2, :],
                            start=(kc == 0), stop=(kc == KC - 1),
                        )
                    nc.vector.tensor_copy(
                        out=vN[:, tt, cc * NCH:(cc + 1) * NCH], in_=ps[:])

            # queue q/k head slabs in consumption order (ring-gated)
            wk_slab = [load_head_slab(wk_d, h) for h in range(N_KV)]
            wq_slab = {}
            for h in range(REP):
                wq_slab[(0, h)] = load_head_slab(wq_d, (0, h))
            for h in range(REP):
                wq_slab[(1, h)] = load_head_slab(wq_d, (1, h))

            # ---- phase 2: k projection (transposed) + RoPE ---------------
            for h in range(N_KV):
                for tc_ in range(TC):
                    ps = ps_pool.tile([P, NCH], F32, tag="mm")
                    proj_T_mms(wk_slab[h], tc_, ps, 0, KC)
                    rope_t(ps, tc_, kT[:, h, tc_ * NCH:(tc_ + 1) * NCH])

            # ---- phase 3: q projection for group 0 -----------------------
            qT_cur = qtg_pool.tile([P, REP, S], BF16, tag="qtg")
            for h in range(REP):
                for tc_ in range(TC):
                    ps = ps_pool.tile([P, NCH], F32, tag="mm")
                    proj_T_mms(wq_slab[(0, h)], tc_, ps, 0, KC)
                    rope_t(ps, tc_, qT_cur[:, h, tc_ * NCH:(tc_ + 1) * NCH])

            # ---- groups 0..6: attention woven with next q projection -----
            for g in range(N_KV - 1):
                qT_next = qtg_pool.tile([P, REP, S], BF16, tag="qtg")
                fillers = [
                    make_proj_filler(
                        wq_slab[(g + 1, i // 2)], i % 2,
                        qT_next[:, i // 2, (i % 2) * NCH:(i % 2 + 1) * NCH])
                    for i in range(8)
                ]
                for i in range(8):
                    if g + 2 <= N_KV - 1 and i in (2, 4, 6, 7):
                        hh = {2: 0, 4: 1, 6: 2, 7: 3}[i]
                        wq_slab[(g + 2, hh)] = load_head_slab(wq_d, (g + 2, hh))
                    emit_unit(g, i // 2, i % 2, qT_cur, fillers[i])
                qT_cur = qT_next

            # ---- group 7 + wo --------------------------------------------
            xres_cm.__exit__(None, None, None)
            ores_cm = tc.tile_pool(name="ores", bufs=1)
            ores = ores_cm.__enter__()
            ot_all = ores.tile([P, N_HEADS, S], BF16)  # [d, h, t]

            def bounce(qc):
                for g in range(7):
                    nc.sync.dma_start(
                        out=ot_all[:, g * REP:(g + 1) * REP,
                                   qc * NCH:(qc + 1) * NCH],
                        in_=otb_d.ap()[qc, :, g * REP:(g + 1) * REP, :],
                    )

            def make_wo_chain(slab, ct, tc_):
                """Closures: 8x(4 wo matmuls over heads, slab stationary)
                + copy/DMA tail writing the transposed output."""
                hold = {}
                clos = []
                for j in range(8):
                    def mm(j=j):
                        if j == 0:
                            hold["ps"] = ps_pool.tile([P, NCH], F32, tag="mm",
                                                      name="wops")
                        for h in range(j * 4, (j + 1) * 4):
                            nc.tensor.matmul(
                                hold["ps"][:],
                                lhsT=slab[:, h, :],
                                rhs=ot_all[:, h, tc_ * NCH:(tc_ + 1) * NCH],
                                start=(h == 0), stop=(h == N_HEADS - 1),
                            )
                    clos.append(mm)
                def tail():
                    outt = out_pool.tile([P, NCH], F32, tag="outp")
                    nc.vector.tensor_copy(out=outt[:], in_=hold["ps"][:])
                    nc.sync.dma_start(
                        out=out_d.ap()[ct * P:(ct + 1) * P,
                                       tc_ * NCH:(tc_ + 1) * NCH],
                        in_=outt[:])
                clos.append(tail)
                return clos

            g7dest = lambda r, qc: ot_all[:, 28 + r, qc * NCH:(qc + 1) * NCH]

            bounce(0)
            bounce(1)
            wo_slab = [load_head_slab(wo_d, ct) for ct in range(3)]
            ch0 = [make_wo_chain(wo_slab[ct], ct, 0) for ct in range(3)]
            emit_unit(7, 0, 0, qT_cur, [], direct_dest=g7dest(0, 0))
            emit_unit(7, 1, 0, qT_cur, ch0[0][0:7], direct_dest=g7dest(1, 0))
            emit_unit(7, 2, 0, qT_cur, ch0[1][0:7], direct_dest=g7dest(2, 0))
            emit_unit(7, 3, 0, qT_cur, ch0[2][0:7], direct_dest=g7dest(3, 0))
            for ct in range(3):
                for cl in ch0[ct][7:]:
                    cl()
            ch1 = [make_wo_chain(wo_slab[ct], ct, 1) for ct in range(3)]
            emit_unit(7, 0, 1, qT_cur, ch1[0][0:7], direct_dest=g7dest(0, 1))
            emit_unit(7, 1, 1, qT_cur, ch1[1][0:7], direct_dest=g7dest(1, 1))
            emit_unit(7, 2, 1, qT_cur, ch1[2][0:7], direct_dest=g7dest(2, 1))
            emit_unit(7, 3, 1, qT_cur, [], direct_dest=g7dest(3, 1))
            for ct in range(3):
                for cl in ch1[ct][7:]:
                    cl()
            for ct in range(3, KC):
                slab = load_head_slab(wo_d, ct)
                for tc_ in range(TC):
                    for cl in make_wo_chain(slab, ct, tc_):
                        cl()
            ores_cm.__exit__(None, None, None)
            kvres_cm.__exit__(None, None, None)

    nc.compile()
    return nc


# host-side input preparation -------------------------------------------------

_ROPE_PERM = np.concatenate([np.arange(0, HEAD_DIM, 2), np.arange(1, HEAD_DIM, 2)])


def _permute_heads(w, n_heads):
    """Permute columns within each head so rotation pairs become
    contiguous (even | odd) halves."""
    w = w.reshape(w.shape[0], n_heads, HEAD_DIM)
    return w[:, :, _ROPE_PERM].reshape(w.shape[0], n_heads * HEAD_DIM)


def _w_layout(w):
    """[DIM, C] f32 -> [C/512, 2, 128, 16, 512] bf16 moving-slab layout."""
    C = w.shape[1]
    wl = w.reshape(2, KH, P, C // NCH, NCH).transpose(3, 0, 2, 1, 4)
    return np.ascontiguousarray(wl).astype(ml_dtypes.bfloat16)


def _prep_shared(cos, sin, wq, wk, wv, wo):
    wq_p = _permute_heads(np.asarray(wq, dtype=np.float32), N_HEADS)
    wk_p = _permute_heads(np.asarray(wk, dtype=np.float32), N_KV)
    # lhsT chunk layouts: [g, h, k-part, kc, d] / [h, k-part, kc, d]
    wq_l = np.ascontiguousarray(
        wq_p.reshape(KC, P, N_KV, REP, HEAD_DIM).transpose(2, 3, 1, 0, 4)
    ).astype(ml_dtypes.bfloat16)
    wk_l = np.ascontiguousarray(
        wk_p.reshape(KC, P, N_KV, HEAD_DIM).transpose(2, 1, 0, 3)
    ).astype(ml_dtypes.bfloat16)
    wv_l = _w_layout(np.asarray(wv, dtype=np.float32))
    # wo lhsT slabs [ct, d, h, c]
    wo_l = np.ascontiguousarray(
        np.asarray(wo, dtype=np.float32)
        .reshape(N_HEADS, HEAD_DIM, KC, P).transpose(2, 1, 0, 3)
    ).astype(ml_dtypes.bfloat16)
    # positions restart at 0 per block, so block 0's tables serve all cores
    c64 = np.asarray(cos[:S], dtype=np.float32).T          # [64, S]
    s64 = np.asarray(sin[:S], dtype=np.float32).T
    c2_l = np.ascontiguousarray(
        np.concatenate([c64, c64], axis=0)).astype(ml_dtypes.bfloat16)
    s2n_l = np.ascontiguousarray(
        np.concatenate([-s64, s64], axis=0)).astype(ml_dtypes.bfloat16)
    return c2_l, s2n_l, wq_l, wk_l, wv_l, wo_l


def _prep_x_block(xb):
    """x block [S, DIM] f32 -> xt [128, KC, S] bf16 (transposed)."""
    xtb = xb.T.reshape(KC, P, S).transpose(1, 0, 2)
    return np.ascontiguousarray(xtb).astype(ml_dtypes.bfloat16)


def kernel(x, cos, sin, wq, wk, wv, wo):
    if "nc" not in _CACHE:
        _CACHE["nc"] = build_nc()
    nc = _CACHE["nc"]

    x = np.asarray(x, dtype=np.float32)
    c2_l, s2n_l, wq_l, wk_l, wv_l, wo_l = _prep_shared(cos, sin, wq, wk, wv, wo)

    in_maps = []
    for b in range(B):
        in_maps.append({
            "xt": _prep_x_block(x[b * S:(b + 1) * S]),
            "c2": c2_l,
            "s2n": s2n_l,
            "wq": wq_l,
            "wk": wk_l,
            "wv": wv_l,
            "wo": wo_l,
        })
    _CACHE["last_in_maps"] = in_maps
    res = run_bass_kernel_spmd(nc, in_maps, core_ids=list(range(B)))
    _CACHE["last_results"] = res
    # per-core output is transposed [DIM, S]
    out = np.concatenate([res.results[b]["out"].T for b in range(B)], axis=0)
    return np.ascontiguousarray(out, dtype=np.float32)


# revision 29
# speedup vs baseline: 1.1354x; 1.0020x over previous
"""Trainium2 Bass kernel for nn_Attention_45999099740384.

GQA attention over 8 independent packed sequences (block-diagonal mask with
equal blocks). Sharding: data-parallel over the 8 blocks - one block of
S=1024 tokens per NeuronCore, weights replicated, zero collectives.

Per-core pipeline (bf16 TensorEngine, fp32 PSUM):
  1. v projection in natural layout (xt stationary, wv moving).
  2. q/k projections TRANSPOSED (weight chunks stationary, xt moving) so
     q^T/k^T come out in [head_dim, t] layout directly - no PE transposes.
  3. RoPE applied in transposed layout on VectorE: host-permuted weight
     columns put rotation pairs into (even|odd) partition halves; the
     cross-partition half-swap is done with two 64-partition copies, then
     two mults against duplicated cos / sign-flipped sin tables and an add.
  4. scores computed transposed: ST[s,q] = kT.T @ qT -> ScalarE exp ->
     P^T tiles in SBUF; P@V needs no transpose of P.
  5. softmax row-sums via a ones[128,128] stationary matmul; reciprocal on
     VectorE (keeps ScalarE exp-only: a single activation table, no
     ACT_TABLE_LOAD churn); normalization deferred to after P@V.
  6. wo matmul from the transposed attention output.

Scheduling: the q projection of group g+1 is woven between the score
matmuls of group g's attention units so the PE never waits for ScalarE
exp; for the last group the leading wo matmul chains are woven in instead.
Attention outputs of groups 0-6 bounce through DRAM (SBUF cannot hold
ot_all while xt is still resident); group 7 writes ot_all directly.
"""

import numpy as np
import ml_dtypes

import concourse.bass as bass
import concourse.mybir as mybir
import concourse.tile as tile
from concourse import bacc
from concourse.bass_utils import run_bass_kernel_spmd

# problem constants (hardcoded per task instructions)
DIM = 4096
N_HEADS = 32
HEAD_DIM = 128
N_KV = 8
REP = 4
B = 8
S = 1024
T = B * S

P = 128                  # SBUF partitions
KC = DIM // P            # 32 contraction chunks of 128
KH = KC // 2             # 16 (w_layout half-chunk count)
TT = S // P              # 8 token tiles of 128
TC = S // 512            # 2 token chunks of 512
NCH = 512                # matmul moving free dim
SCALE = HEAD_DIM ** -0.5

F32 = mybir.dt.float32
BF16 = mybir.dt.bfloat16

_CACHE = {}


def build_nc():
    nc = bacc.Bacc("TRN2", target_bir_lowering=False, debug=False, num_devices=8)

    xt_d = nc.dram_tensor("xt", [P, KC, S], BF16, kind="ExternalInput")
    c2_d = nc.dram_tensor("c2", [P, S], BF16, kind="ExternalInput")
    s2n_d = nc.dram_tensor("s2n", [P, S], BF16, kind="ExternalInput")
    # wq: [g, h, k-part, kc, d] lhsT chunks (rope-permuted d columns)
    wq_d = nc.dram_tensor("wq", [N_KV, REP, P, KC, HEAD_DIM], BF16,
                          kind="ExternalInput")
    wk_d = nc.dram_tensor("wk", [N_KV, P, KC, HEAD_DIM], BF16,
                          kind="ExternalInput")
    # wv/wo: moving-operand slabs [chunk, half, 128, 16, 512]
    wv_d = nc.dram_tensor("wv", [2, 2, P, KH, NCH], BF16, kind="ExternalInput")
    # wo: [ct, d, h, c] lhsT slabs (stationary, streamed per 128-col tile)
    wo_d = nc.dram_tensor("wo", [KC, P, N_HEADS, P], BF16, kind="ExternalInput")
    # transposed output [DIM, S]; host untransposes
    out_d = nc.dram_tensor("out", [DIM, S], F32, kind="ExternalOutput")
    # attention-output bounce for groups 0..6
    otb_d = nc.dram_tensor("otb", [2, P, 28, NCH], BF16)

    with tile.TileContext(nc) as tc:
        with (
            tc.tile_pool(name="const", bufs=1) as const,
            tc.tile_pool(name="wpool", bufs=1) as wpool,
            tc.tile_pool(name="qtg", bufs=2) as qtg_pool,
            tc.tile_pool(name="ptp", bufs=2) as pt_pool,
            tc.tile_pool(name="scr", bufs=3) as scr_pool,
            tc.tile_pool(name="rcbp", bufs=1) as rcb_pool,
            tc.tile_pool(name="otp", bufs=1) as ot_pool,
            tc.tile_pool(name="outp", bufs=2) as out_pool,
            tc.tile_pool(name="pt8p", bufs=1) as pt8_pool,
            tc.tile_pool(name="psmm", bufs=2, space="PSUM") as ps_pool,
            tc.tile_pool(name="psst", bufs=3, space="PSUM") as st_pool,
            tc.tile_pool(name="psor", bufs=3, space="PSUM") as or_pool,
        ):
            ones8 = const.tile([P, 2, P], mybir.dt.float8e5)
            nc.vector.memset(ones8[:], 1.0)
            # exp bias -ln(128): keeps exp within fp8e4 range for the fp8
            # row-sum copy; cancels in the softmax normalization
            ebias = const.tile([P, 1], F32)
            nc.vector.memset(ebias[:], -4.852030263919617)

            kvres_cm = tc.tile_pool(name="kvres", bufs=1)
            kvres = kvres_cm.__enter__()
            kT = kvres.tile([P, N_KV, S], BF16)              # [d, kv, t]
            vN = kvres.tile([P, TT, N_KV * HEAD_DIM], BF16)  # [s, s_tile, kv*d]

            xres_cm = tc.tile_pool(name="xres", bufs=1)
            xres = xres_cm.__enter__()
            xt = xres.tile([P, KC, S], BF16)

            # ---- weight streaming helpers --------------------------------
            def load_head_slab(w_dram, idx):
                """[P, KC, 128] lhsT slab for one q/k head (2 DMAs)."""
                sl = wpool.tile([P, KC, HEAD_DIM], BF16, tag="wst", bufs=3)
                src = w_dram.ap()[idx] if isinstance(idx, int) \
                    else w_dram.ap()[idx[0], idx[1]]
                nc.sync.dma_start(out=sl[:, 0:KH, :], in_=src[:, 0:KH, :])
                nc.sync.dma_start(out=sl[:, KH:KC, :], in_=src[:, KH:KC, :])
                return sl

            # ---- startup DMAs (xt slices interleaved with wv slabs) ------
            def load_sixteenth(w_dram, cc, e):
                sl = wpool.tile([P, 2, NCH], BF16, tag="w16", bufs=17,
                                name="w16t")
                nc.sync.dma_start(
                    out=sl[:],
                    in_=w_dram.ap()[cc, e // 8, :,
                                    (e % 8) * 2:(e % 8) * 2 + 2, :])
                return sl

            def xslice(ts):
                nc.sync.dma_start(out=xt[:, :, ts * P:(ts + 1) * P],
                                  in_=xt_d.ap()[:, :, ts * P:(ts + 1) * P])

            wv_e = [[], []]
            wv_e[0].append(load_sixteenth(wv_d, 0, 0))
            for kq in range(4):
                nc.sync.dma_start(out=xt[:, kq * 8:(kq + 1) * 8, 0:P],
                                  in_=xt_d.ap()[:, kq * 8:(kq + 1) * 8, 0:P])
            for e in range(1, 6):
                wv_e[0].append(load_sixteenth(wv_d, 0, e))
            xslice(1)
            for e in range(6, 11):
                wv_e[0].append(load_sixteenth(wv_d, 0, e))
            xslice(2)
            for e in range(11, 16):
                wv_e[0].append(load_sixteenth(wv_d, 0, e))
            xslice(3)
            for e in range(8):
                wv_e[1].append(load_sixteenth(wv_d, 1, e))
            xslice(4)
            xslice(5)
            for e in range(8, 16):
                wv_e[1].append(load_sixteenth(wv_d, 1, e))
            xslice(6)
            xslice(7)
            c2 = const.tile([P, S], BF16)
            nc.sync.dma_start(out=c2[:], in_=c2_d.ap())
            s2n = const.tile([P, S], BF16)
            nc.sync.dma_start(out=s2n[:], in_=s2n_d.ap())

            # ---- compute helpers -----------------------------------------
            def proj_T_mms(slab, tc_, ps, k0, k1):
                """Transposed projection: out[d, t] += slab[kc].T @ xt."""
                for kc in range(k0, k1):
                    nc.tensor.matmul(
                        ps[:],
                        lhsT=slab[:, kc, :],
                        rhs=xt[:, kc, tc_ * NCH:(tc_ + 1) * NCH],
                        start=(kc == 0),
                        stop=(kc == KC - 1),
                    )

            def rope_t(ps, tc_, dest):
                """RoPE in [d, t] layout: dest = ps*c2 + swap_halves(ps)*s2n."""
                sw = scr_pool.tile([P, NCH], F32, tag="scr")
                nc.vector.tensor_copy(out=sw[0:64, :], in_=ps[64:P, :])
                nc.vector.tensor_copy(out=sw[64:P, :], in_=ps[0:64, :])
                m1 = scr_pool.tile([P, NCH], F32, tag="scr")
                nc.vector.tensor_tensor(m1[:], ps[:],
                                        c2[:, tc_ * NCH:(tc_ + 1) * NCH],
                                        mybir.AluOpType.mult)
                m2 = scr_pool.tile([P, NCH], F32, tag="scr")
                nc.vector.tensor_tensor(m2[:], sw[:],
                                        s2n[:, tc_ * NCH:(tc_ + 1) * NCH],
                                        mybir.AluOpType.mult)
                nc.vector.tensor_tensor(dest, m1[:], m2[:],
                                        mybir.AluOpType.add)

            def make_proj_filler(slab, tc_, dest):
                """Closures: 8x(4 proj matmuls) + rope. First closure
                allocates the psum tile."""
                hold = {}
                clos = []
                for j in range(8):
                    def mm(j=j):
                        if j == 0:
                            hold["ps"] = ps_pool.tile([P, NCH], F32, tag="mm",
                                                      name="pjps")
                        proj_T_mms(slab, tc_, hold["ps"], j * 4, (j + 1) * 4)
                    clos.append(mm)
                def rope():
                    rope_t(hold["ps"], tc_, dest)
                clos.append(rope)
                return clos

            def emit_unit(g, r, qc, qT_use, filler, direct_dest=None):
                """One attention unit (kv-group g, q-head r, q-col chunk qc)
                with PE filler closures woven between score matmuls."""
                pt = pt_pool.tile([P, TT, NCH], BF16, tag="pt")

                def score(st):
                    sps = st_pool.tile([P, NCH], F32, tag="st")
                    nc.tensor.matmul(
                        sps[:],
                        lhsT=kT[:, g, st * P:(st + 1) * P],
                        rhs=qT_use[:, r, qc * NCH:(qc + 1) * NCH],
                        start=True, stop=True,
                    )
                    nc.scalar.activation(
                        pt[:, st, :], sps[:],
                        mybir.ActivationFunctionType.Exp, scale=SCALE,
                        bias=ebias[:],
                    )

                fi = 0
                score(0)
                score(1)
                for st in range(2, TT):
                    if fi < len(filler):
                        filler[fi]()
                        fi += 1
                    score(st)
                # fp8 copy of P^T for the double-row row-sum matmuls
                pt8 = pt8_pool.tile([P, TT, NCH], mybir.dt.float8e5, tag="pt8")
                nc.vector.tensor_copy(out=pt8[:], in_=pt[:])
                while fi < len(filler):
                    filler[fi]()
                    fi += 1

                ops = or_pool.tile([P, NCH], F32, tag="or")
                rps = or_pool.tile([P, NCH], F32, tag="or")
                for st in range(TT):
                    nc.tensor.matmul(
                        ops[:], lhsT=vN[:, st, g * P:(g + 1) * P],
                        rhs=pt[:, st, :], start=(st == 0), stop=(st == TT - 1),
                    )
                for j in range(TT // 2):
                    nc.tensor.matmul(
                        rps[:], lhsT=ones8[:],
                        rhs=pt8[:, 2 * j:2 * j + 2, :],
                        start=(j == 0), stop=(j == TT // 2 - 1),
                        perf_mode=mybir.MatmulPerfMode.DoubleRow,
                    )
                rcb = rcb_pool.tile([P, NCH], F32, tag="rcb")
                nc.vector.reciprocal_approx_fast(out=rcb[:], in_=rps[:])
                if direct_dest is not None:
                    nc.vector.tensor_tensor(direct_dest, ops[:], rcb[:],
                                            mybir.AluOpType.mult)
                else:
                    ot = ot_pool.tile([P, NCH], BF16, tag="ot")
                    nc.vector.tensor_tensor(ot[:], ops[:], rcb[:],
                                            mybir.AluOpType.mult)
                    nc.sync.dma_start(out=otb_d.ap()[qc, :, g * REP + r, :],
                                      in_=ot[:])

            # ---- phase 1: v projection (natural layout) ------------------
            for cc in range(2):
                sixteenths = wv_e[cc]
                for tt in range(TT):
                    ps = ps_pool.tile([P, NCH], F32, tag="mm")
                    for kc in range(KC):
                        nc.tensor.matmul(
                            ps[:],
                            lhsT=xt[:, kc, tt * P:(tt + 1) * P],
                            rhs=sixteenths[kc // 2][:, kc % 2, :],
                            start=(kc == 0), stop=(kc == KC - 1),
                        )
                    nc.vector.tensor_copy(
                        out=vN[:, tt, cc * NCH:(cc + 1) * NCH], in_=ps[:])

            # queue q/k head slabs in consumption order (ring-gated)
            wk_slab = [load_head_slab(wk_d, h) for h in range(N_KV)]
            wq_slab = {}
            for h in range(REP):
                wq_slab[(0, h)] = load_head_slab(wq_d, (0, h))
            for h in range(REP):
                wq_slab[(1, h)] = load_head_slab(wq_d, (1, h))

            # ---- phase 2: k projection (transposed) + RoPE ---------------
            for h in range(N_KV):
                for tc_ in range(TC):
                    ps = ps_pool.tile([P, NCH], F32, tag="mm")
                    proj_T_mms(wk_slab[h], tc_, ps, 0, KC)
                    rope_t(ps, tc_, kT[:, h, tc_ * NCH:(tc_ + 1) * NCH])

            # ---- phase 3: q projection for group 0 -----------------------
            qT_cur = qtg_pool.tile([P, REP, S], BF16, tag="qtg")
            for h in range(REP):
                for tc_ in range(TC):
                    ps = ps_pool.tile([P, NCH], F32, tag="mm")
                    proj_T_mms(wq_slab[(0, h)], tc_, ps, 0, KC)
                    rope_t(ps, tc_, qT_cur[:, h, tc_ * NCH:(tc_ + 1) * NCH])

            # ---- groups 0..6: attention woven with next q projection -----
            for g in range(N_KV - 1):
                qT_next = qtg_pool.tile([P, REP, S], BF16, tag="qtg")
                fillers = [
                    make_proj_filler(
                        wq_slab[(g + 1, i // 2)], i % 2,
                        qT_next[:, i // 2, (i % 2) * NCH:(i % 2 + 1) * NCH])
                    for i in range(8)
                ]
                for i in range(8):
                    if g + 2 <= N_KV - 1 and i in (2, 4, 6, 7):
                        hh = {2: 0, 4: 1, 6: 2, 7: 3}[i]
                        wq_slab[(g + 2, hh)] = load_head_slab(wq_d, (g + 2, hh))
                    emit_unit(g, i // 2, i % 2, qT_cur, fillers[i])
                qT_cur = qT_next

            # ---- group 7 + wo --------------------------------------------
            xres_cm.__exit__(None, None, None)
            ores_cm = tc.tile_pool(name="ores", bufs=1)
            ores = ores_cm.__enter__()
            ot_all = ores.tile([P, N_HEADS, S], BF16)  # [d, h, t]

            def bounce(qc):
                for g in range(7):
                    nc.sync.dma_start(
                        out=ot_all[:, g * REP:(g + 1) * REP,
                                   qc * NCH:(qc + 1) * NCH],
                        in_=otb_d.ap()[qc, :, g * REP:(g + 1) * REP, :],
                    )

            def make_wo_chain(slab, ct, tc_):
                """Closures: 8x(4 wo matmuls over heads, slab stationary)
                + copy/DMA tail writing the transposed output."""
                hold = {}
                clos = []
                for j in range(8):
                    def mm(j=j):
                        if j == 0:
                            hold["ps"] = ps_pool.tile([P, NCH], F32, tag="mm",
                                                      name="wops")
                        for h in range(j * 4, (j + 1) * 4):
                            nc.tensor.matmul(
                                hold["ps"][:],
                                lhsT=slab[:, h, :],
                                rhs=ot_all[:, h, tc_ * NCH:(tc_ + 1) * NCH],
                                start=(h == 0), stop=(h == N_HEADS - 1),
                            )
                    clos.append(mm)
                def tail():
                    outt = out_pool.tile([P, NCH], F32, tag="outp")
                    nc.vector.tensor_copy(out=outt[:], in_=hold["ps"][:])
                    nc.sync.dma_start(
                        out=out_d.ap()[ct * P:(ct + 1) * P,
                                       tc_ * NCH:(tc_ + 1) * NCH],
                        in_=outt[:])
                clos.append(tail)
                return clos

            g7dest = lambda r, qc: ot_all[:, 28 + r, qc * NCH:(qc + 1) * NCH]

            bounce(0)
            bounce(1)
            wo_slab = [load_head_slab(wo_d, ct) for ct in range(3)]
            ch0 = [make_wo_chain(wo_slab[ct], ct, 0) for ct in range(3)]
            emit_unit(7, 0, 0, qT_cur, [], direct_dest=g7dest(0, 0))
            emit_unit(7, 1, 0, qT_cur, ch0[0][0:7], direct_dest=g7dest(1, 0))
            emit_unit(7, 2, 0, qT_cur, ch0[1][0:7], direct_dest=g7dest(2, 0))
            emit_unit(7, 3, 0, qT_cur, ch0[2][0:7], direct_dest=g7dest(3, 0))
            for ct in range(3):
                for cl in ch0[ct][7:]:
                    cl()
            ch1 = [make_wo_chain(wo_slab[ct], ct, 1) for ct in range(3)]
            emit_unit(7, 0, 1, qT_cur, ch1[0][0:7], direct_dest=g7dest(0, 1))
            emit_unit(7, 1, 1, qT_cur, ch1[1][0:7], direct_dest=g7dest(1, 1))
            emit_unit(7, 2, 1, qT_cur, ch1[2][0:7], direct_dest=g7dest(2, 1))
            emit_unit(7, 3, 1, qT_cur, [], direct_dest=g7dest(3, 1))
            for ct in range(3):
                for cl in ch1[ct][7:]:
                    cl()
            for ct in range(3, KC):
                slab = load_head_slab(wo_d, ct)
                for tc_ in range(TC):
                    for cl in make_wo_chain(slab, ct, tc_):
                        cl()
            ores_cm.__exit__(None, None, None)
            kvres_cm.__exit__(None, None, None)

    nc.compile()
    return nc


# host-side input preparation -------------------------------------------------

_ROPE_PERM = np.concatenate([np.arange(0, HEAD_DIM, 2), np.arange(1, HEAD_DIM, 2)])


def _permute_heads(w, n_heads):
    """Permute columns within each head so rotation pairs become
    contiguous (even | odd) halves."""
    w = w.reshape(w.shape[0], n_heads, HEAD_DIM)
    return w[:, :, _ROPE_PERM].reshape(w.shape[0], n_heads * HEAD_DIM)


def _w_layout(w):
    """[DIM, C] f32 -> [C/512, 2, 128, 16, 512] bf16 moving-slab layout."""
    C = w.shape[1]
    wl = w.reshape(2, KH, P, C // NCH, NCH).transpose(3, 0, 2, 1, 4)
    return np.ascontiguousarray(wl).astype(ml_dtypes.bfloat16)


def _prep_shared(cos, sin, wq, wk, wv, wo):
    wq_p = _permute_heads(np.asarray(wq, dtype=np.float32), N_HEADS)
    wk_p = _permute_heads(np.asarray(wk, dtype=np.float32), N_KV)
    # lhsT chunk layouts: [g, h, k-part, kc, d] / [h, k-part, kc, d]
    wq_l = np.ascontiguousarray(
        wq_p.reshape(KC, P, N_KV, REP, HEAD_DIM).transpose(2, 3, 1, 0, 4)
    ).astype(ml_dtypes.bfloat16)
    wk_l = np.ascontiguousarray(
        wk_p.reshape(KC, P, N_KV, HEAD_DIM).transpose(2, 1, 0, 3)
    ).astype(ml_dtypes.bfloat16)
    wv_l = _w_layout(np.asarray(wv, dtype=np.float32))
    # wo lhsT slabs [ct, d, h, c]
    wo_l = np.ascontiguousarray(
        np.asarray(wo, dtype=np.float32)
        .reshape(N_HEADS, HEAD_DIM, KC, P).transpose(2, 1, 0, 3)
    ).astype(ml_dtypes.bfloat16)
    # positions restart at 0 per block, so block 0's tables serve all cores
    c64 = np.asarray(cos[:S], dtype=np.float32).T          # [64, S]
    s64 = np.asarray(sin[:S], dtype=np.float32).T
    c2_l = np.ascontiguousarray(
        np.concatenate([c64, c64], axis=0)).astype(ml_dtypes.bfloat16)
    s2n_l = np.ascontiguousarray(
        np.concatenate([-s64, s64], axis=0)).astype(ml_dtypes.bfloat16)
    return c2_l, s2n_l, wq_l, wk_l, wv_l, wo_l


def _prep_x_block(xb):
    """x block [S, DIM] f32 -> xt [128, KC, S] bf16 (transposed)."""
    xtb = xb.T.reshape(KC, P, S).transpose(1, 0, 2)
    return np.ascontiguousarray(xtb).astype(ml_dtypes.bfloat16)


def kernel(x, cos, sin, wq, wk, wv, wo):
    if "nc" not in _CACHE:
        _CACHE["nc"] = build_nc()
    nc = _CACHE["nc"]

    x = np.asarray(x, dtype=np.float32)
    c2_l, s2n_l, wq_l, wk_l, wv_l, wo_l = _prep_shared(cos, sin, wq, wk, wv, wo)

    in_maps = []
    for b in range(B):
        in_maps.append({
            "xt": _prep_x_block(x[b * S:(b + 1) * S]),
            "c2": c2_l,
            "s2n": s2n_l,
            "wq": wq_l,
            "wk": wk_l,
            "wv": wv_l,
            "wo": wo_l,
        })
    _CACHE["last_in_maps"] = in_maps
    res = run_bass_kernel_spmd(nc, in_maps, core_ids=list(range(B)))
    _CACHE["last_results"] = res
    # per-core output is transposed [DIM, S]
    out = np.concatenate([res.results[b]["out"].T for b in range(B)], axis=0)
    return np.ascontiguousarray(out, dtype=np.float32)


# revision 35
# speedup vs baseline: 1.1378x; 1.0021x over previous
"""Trainium2 Bass kernel for nn_Attention_45999099740384.

GQA attention over 8 independent packed sequences (block-diagonal mask with
equal blocks). Sharding: data-parallel over the 8 blocks - one block of
S=1024 tokens per NeuronCore, weights replicated, zero collectives.

Per-core pipeline (bf16 TensorEngine, fp32 PSUM):
  1. v projection in natural layout (xt stationary, wv moving).
  2. q/k projections TRANSPOSED (weight chunks stationary, xt moving) so
     q^T/k^T come out in [head_dim, t] layout directly - no PE transposes.
  3. RoPE applied in transposed layout on VectorE: host-permuted weight
     columns put rotation pairs into (even|odd) partition halves; the
     cross-partition half-swap is done with two 64-partition copies, then
     two mults against duplicated cos / sign-flipped sin tables and an add.
  4. scores computed transposed: ST[s,q] = kT.T @ qT -> ScalarE exp ->
     P^T tiles in SBUF; P@V needs no transpose of P.
  5. softmax row-sums via a ones[128,128] stationary matmul; reciprocal on
     VectorE (keeps ScalarE exp-only: a single activation table, no
     ACT_TABLE_LOAD churn); normalization deferred to after P@V.
  6. wo matmul from the transposed attention output.

Scheduling: the q projection of group g+1 is woven between the score
matmuls of group g's attention units so the PE never waits for ScalarE
exp; for the last group the leading wo matmul chains are woven in instead.
Attention outputs of groups 0-6 bounce through DRAM (SBUF cannot hold
ot_all while xt is still resident); group 7 writes ot_all directly.
"""

import numpy as np
import ml_dtypes

import concourse.bass as bass
import concourse.mybir as mybir
import concourse.tile as tile
from concourse import bacc
from concourse.bass_utils import run_bass_kernel_spmd

# problem constants (hardcoded per task instructions)
DIM = 4096
N_HEADS = 32
HEAD_DIM = 128
N_KV = 8
REP = 4
B = 8
S = 1024
T = B * S

P = 128                  # SBUF partitions
KC = DIM // P            # 32 contraction chunks of 128
KH = KC // 2             # 16 (w_layout half-chunk count)
TT = S // P              # 8 token tiles of 128
TC = S // 512            # 2 token chunks of 512
NCH = 512                # matmul moving free dim
SCALE = HEAD_DIM ** -0.5

F32 = mybir.dt.float32
BF16 = mybir.dt.bfloat16

_CACHE = {}


def build_nc():
    nc = bacc.Bacc("TRN2", target_bir_lowering=False, debug=False, num_devices=8)

    # xt slice-major: [t-slice, k-part, kc, t-within-slice] so slice DMAs are
    # contiguous 8KB/partition (strided layout produced 256B DMA packets)
    xt_d = nc.dram_tensor("xt", [TT, P, KC, P], BF16, kind="ExternalInput")
    c2_d = nc.dram_tensor("c2", [P, S], BF16, kind="ExternalInput")
    s2n_d = nc.dram_tensor("s2n", [P, S], BF16, kind="ExternalInput")
    # wq: [g, h, k-part, kc, d] lhsT chunks (rope-permuted d columns)
    wq_d = nc.dram_tensor("wq", [N_KV, REP, P, KC, HEAD_DIM], BF16,
                          kind="ExternalInput")
    wk_d = nc.dram_tensor("wk", [N_KV, P, KC, HEAD_DIM], BF16,
                          kind="ExternalInput")
    # wv/wo: moving-operand slabs [chunk, half, 128, 16, 512]
    wv_d = nc.dram_tensor("wv", [2, 2, P, KH, NCH], BF16, kind="ExternalInput")
    # wo: [ct, d, h, c] lhsT slabs (stationary, streamed per 128-col tile)
    wo_d = nc.dram_tensor("wo", [KC, P, N_HEADS, P], BF16, kind="ExternalInput")
    # transposed output [DIM, S]; host untransposes
    out_d = nc.dram_tensor("out", [DIM, S], F32, kind="ExternalOutput")
    # attention-output bounce for groups 0..6
    otb_d = nc.dram_tensor("otb", [2, P, 28, NCH], BF16)

    with tile.TileContext(nc) as tc:
        with (
            tc.tile_pool(name="const", bufs=1) as const,
            tc.tile_pool(name="wpool", bufs=1) as wpool,
            tc.tile_pool(name="qtg", bufs=2) as qtg_pool,
            tc.tile_pool(name="ptp", bufs=2) as pt_pool,
            tc.tile_pool(name="scr", bufs=3) as scr_pool,
            tc.tile_pool(name="rcbp", bufs=1) as rcb_pool,
            tc.tile_pool(name="otp", bufs=1) as ot_pool,
            tc.tile_pool(name="outp", bufs=2) as out_pool,
            tc.tile_pool(name="pt8p", bufs=1) as pt8_pool,
            tc.tile_pool(name="psmm", bufs=2, space="PSUM") as ps_pool,
            tc.tile_pool(name="psst", bufs=3, space="PSUM") as st_pool,
            tc.tile_pool(name="psor", bufs=3, space="PSUM") as or_pool,
        ):
            ones8 = const.tile([P, 2, P], mybir.dt.float8e5)
            nc.vector.memset(ones8[:], 1.0)
            # exp bias -ln(128): keeps exp within fp8e4 range for the fp8
            # row-sum copy; cancels in the softmax normalization
            ebias = const.tile([P, 1], F32)
            nc.vector.memset(ebias[:], -4.852030263919617)

            kvres_cm = tc.tile_pool(name="kvres", bufs=1)
            kvres = kvres_cm.__enter__()
            kT = kvres.tile([P, N_KV, S], BF16)              # [d, kv, t]
            vN = kvres.tile([P, TT, N_KV * HEAD_DIM], BF16)  # [s, s_tile, kv*d]

            xres_cm = tc.tile_pool(name="xres", bufs=1)
            xres = xres_cm.__enter__()
            xt = xres.tile([P, TT, KC, P], BF16)  # [k-part, slice, kc, tj]

            # ---- weight streaming helpers --------------------------------
            def load_head_slab(w_dram, idx):
                """[P, KC, 128] lhsT slab for one q/k head (2 DMAs)."""
                sl = wpool.tile([P, KC, HEAD_DIM], BF16, tag="wst", bufs=3)
                src = w_dram.ap()[idx] if isinstance(idx, int) \
                    else w_dram.ap()[idx[0], idx[1]]
                nc.sync.dma_start(out=sl[:, 0:KH, :], in_=src[:, 0:KH, :])
                nc.sync.dma_start(out=sl[:, KH:KC, :], in_=src[:, KH:KC, :])
                return sl

            # ---- startup DMAs (xt slices interleaved with wv slabs) ------
            def load_sixteenth(w_dram, cc, e):
                sl = wpool.tile([P, 2, NCH], BF16, tag="w16", bufs=17,
                                name="w16t")
                nc.sync.dma_start(
                    out=sl[:],
                    in_=w_dram.ap()[cc, e // 8, :,
                                    (e % 8) * 2:(e % 8) * 2 + 2, :])
                return sl

            def xslice(ts):
                nc.sync.dma_start(out=xt[:, ts, 0:KH, :],
                                  in_=xt_d.ap()[ts, :, 0:KH, :])
                nc.sync.dma_start(out=xt[:, ts, KH:KC, :],
                                  in_=xt_d.ap()[ts, :, KH:KC, :])

            wv_e = [[], []]
            wv_e[0].append(load_sixteenth(wv_d, 0, 0))
            xslice(0)
            for e in range(1, 6):
                wv_e[0].append(load_sixteenth(wv_d, 0, e))
            xslice(1)
            for e in range(6, 11):
                wv_e[0].append(load_sixteenth(wv_d, 0, e))
            xslice(2)
            for e in range(11, 16):
                wv_e[0].append(load_sixteenth(wv_d, 0, e))
            xslice(3)
            for e in range(8):
                wv_e[1].append(load_sixteenth(wv_d, 1, e))
            xslice(4)
            xslice(5)
            for e in range(8, 16):
                wv_e[1].append(load_sixteenth(wv_d, 1, e))
            xslice(6)
            xslice(7)
            c2 = const.tile([P, S], BF16)
            nc.sync.dma_start(out=c2[:], in_=c2_d.ap())
            s2n = const.tile([P, S], BF16)
            nc.sync.dma_start(out=s2n[:], in_=s2n_d.ap())

            # ---- compute helpers -----------------------------------------
            def proj_T_mms(slab, tc_, ps, k0, k1):
                """Transposed projection: out[d, t] += slab[kc].T @ xt."""
                for kc in range(k0, k1):
                    nc.tensor.matmul(
                        ps[:],
                        lhsT=slab[:, kc, :],
                        rhs=xt[:, 4 * tc_:4 * tc_ + 4, kc, :],
                        start=(kc == 0),
                        stop=(kc == KC - 1),
                    )

            def rope_t(ps, tc_, dest):
                """RoPE in [d, t] layout: dest = ps*c2 + swap_halves(ps)*s2n."""
                sw = scr_pool.tile([P, NCH], F32, tag="scr")
                nc.vector.tensor_copy(out=sw[0:64, :], in_=ps[64:P, :])
                nc.vector.tensor_copy(out=sw[64:P, :], in_=ps[0:64, :])
                m1 = scr_pool.tile([P, NCH], F32, tag="scr")
                nc.vector.tensor_tensor(m1[:], ps[:],
                                        c2[:, tc_ * NCH:(tc_ + 1) * NCH],
                                        mybir.AluOpType.mult)
                m2 = scr_pool.tile([P, NCH], F32, tag="scr")
                nc.vector.tensor_tensor(m2[:], sw[:],
                                        s2n[:, tc_ * NCH:(tc_ + 1) * NCH],
                                        mybir.AluOpType.mult)
                nc.vector.tensor_tensor(dest, m1[:], m2[:],
                                        mybir.AluOpType.add)

            def make_proj_filler(slab, tc_, dest):
                """Closures: 8x(4 proj matmuls) + rope. First closure
                allocates the psum tile."""
                hold = {}
                clos = []
                for j in range(8):
                    def mm(j=j):
                        if j == 0:
                            hold["ps"] = ps_pool.tile([P, NCH], F32, tag="mm",
                                                      name="pjps")
                        proj_T_mms(slab, tc_, hold["ps"], j * 4, (j + 1) * 4)
                    clos.append(mm)
                def rope():
                    rope_t(hold["ps"], tc_, dest)
                clos.append(rope)
                return clos

            def emit_unit(g, r, qc, qT_use, filler, direct_dest=None):
                """One attention unit (kv-group g, q-head r, q-col chunk qc)
                with PE filler closures woven between score matmuls."""
                pt = pt_pool.tile([P, TT, NCH], BF16, tag="pt")

                def score(st):
                    sps = st_pool.tile([P, NCH], F32, tag="st")
                    nc.tensor.matmul(
                        sps[:],
                        lhsT=kT[:, g, st * P:(st + 1) * P],
                        rhs=qT_use[:, r, qc * NCH:(qc + 1) * NCH],
                        start=True, stop=True,
                    )
                    nc.scalar.activation(
                        pt[:, st, :], sps[:],
                        mybir.ActivationFunctionType.Exp, scale=SCALE,
                        bias=ebias[:],
                    )

                fi = 0
                score(0)
                score(1)
                for st in range(2, TT):
                    if fi < len(filler):
                        filler[fi]()
                        fi += 1
                    score(st)
                # fp8 copy of P^T for the double-row row-sum matmuls
                pt8 = pt8_pool.tile([P, TT, NCH], mybir.dt.float8e5, tag="pt8")
                nc.vector.tensor_copy(out=pt8[:], in_=pt[:])
                while fi < len(filler):
                    filler[fi]()
                    fi += 1

                ops = or_pool.tile([P, NCH], F32, tag="or")
                rps = or_pool.tile([P, NCH], F32, tag="or")
                for st in range(TT):
                    nc.tensor.matmul(
                        ops[:], lhsT=vN[:, st, g * P:(g + 1) * P],
                        rhs=pt[:, st, :], start=(st == 0), stop=(st == TT - 1),
                    )
                for j in range(TT // 2):
                    nc.tensor.matmul(
                        rps[:], lhsT=ones8[:],
                        rhs=pt8[:, 2 * j:2 * j + 2, :],
                        start=(j == 0), stop=(j == TT // 2 - 1),
                        perf_mode=mybir.MatmulPerfMode.DoubleRow,
                    )
                rcb = rcb_pool.tile([P, NCH], F32, tag="rcb")
                nc.vector.reciprocal_approx_fast(out=rcb[:], in_=rps[:])
                if direct_dest is not None:
                    nc.vector.tensor_tensor(direct_dest, ops[:], rcb[:],
                                            mybir.AluOpType.mult)
                else:
                    ot = ot_pool.tile([P, NCH], BF16, tag="ot")
                    nc.vector.tensor_tensor(ot[:], ops[:], rcb[:],
                                            mybir.AluOpType.mult)
                    nc.sync.dma_start(out=otb_d.ap()[qc, :, g * REP + r, :],
                                      in_=ot[:])

            # ---- phase 1: v projection (natural layout) ------------------
            for cc in range(2):
                sixteenths = wv_e[cc]
                for tt in range(TT):
                    ps = ps_pool.tile([P, NCH], F32, tag="mm")
                    for kc in range(KC):
                        nc.tensor.matmul(
                            ps[:],
                            lhsT=xt[:, tt, kc, :],
                            rhs=sixteenths[kc // 2][:, kc % 2, :],
                            start=(kc == 0), stop=(kc == KC - 1),
                        )
                    nc.vector.tensor_copy(
                        out=vN[:, tt, cc * NCH:(cc + 1) * NCH], in_=ps[:])

            # queue q/k head slabs in consumption order (ring-gated)
            wk_slab = [load_head_slab(wk_d, h) for h in range(N_KV)]
            wq_slab = {}
            for h in range(REP):
                wq_slab[(0, h)] = load_head_slab(wq_d, (0, h))
            for h in range(REP):
                wq_slab[(1, h)] = load_head_slab(wq_d, (1, h))

            # ---- phase 2: k projection (transposed) + RoPE ---------------
            for h in range(N_KV):
                for tc_ in range(TC):
                    ps = ps_pool.tile([P, NCH], F32, tag="mm")
                    proj_T_mms(wk_slab[h], tc_, ps, 0, KC)
                    rope_t(ps, tc_, kT[:, h, tc_ * NCH:(tc_ + 1) * NCH])

            # ---- phase 3: q projection for group 0 -----------------------
            qT_cur = qtg_pool.tile([P, REP, S], BF16, tag="qtg")
            for h in range(REP):
                for tc_ in range(TC):
                    ps = ps_pool.tile([P, NCH], F32, tag="mm")
                    proj_T_mms(wq_slab[(0, h)], tc_, ps, 0, KC)
                    rope_t(ps, tc_, qT_cur[:, h, tc_ * NCH:(tc_ + 1) * NCH])

            # ---- groups 0..6: attention woven with next q projection -----
            for g in range(N_KV - 1):
                qT_next = qtg_pool.tile([P, REP, S], BF16, tag="qtg")
                fillers = [
                    make_proj_filler(
                        wq_slab[(g + 1, i // 2)], i % 2,
                        qT_next[:, i // 2, (i % 2) * NCH:(i % 2 + 1) * NCH])
                    for i in range(8)
                ]
                for i in range(8):
                    if g + 2 <= N_KV - 1 and i in (2, 4, 6, 7):
                        hh = {2: 0, 4: 1, 6: 2, 7: 3}[i]
                        wq_slab[(g + 2, hh)] = load_head_slab(wq_d, (g + 2, hh))
                    emit_unit(g, i // 2, i % 2, qT_cur, fillers[i])
                qT_cur = qT_next

            # ---- group 7 + wo --------------------------------------------
            xres_cm.__exit__(None, None, None)
            ores_cm = tc.tile_pool(name="ores", bufs=1)
            ores = ores_cm.__enter__()
            ot_all = ores.tile([P, N_HEADS, S], BF16)  # [d, h, t]

            def bounce(qc):
                for g in range(7):
                    nc.sync.dma_start(
                        out=ot_all[:, g * REP:(g + 1) * REP,
                                   qc * NCH:(qc + 1) * NCH],
                        in_=otb_d.ap()[qc, :, g * REP:(g + 1) * REP, :],
                    )

            def make_wo_chain(slab, ct, tc_):
                """Closures: 8x(4 wo matmuls over heads, slab stationary)
                + copy/DMA tail writing the transposed output."""
                hold = {}
                clos = []
                for j in range(8):
                    def mm(j=j):
                        if j == 0:
                            hold["ps"] = ps_pool.tile([P, NCH], F32, tag="mm",
                                                      name="wops")
                        for h in range(j * 4, (j + 1) * 4):
                            nc.tensor.matmul(
                                hold["ps"][:],
                                lhsT=slab[:, h, :],
                                rhs=ot_all[:, h, tc_ * NCH:(tc_ + 1) * NCH],
                                start=(h == 0), stop=(h == N_HEADS - 1),
                            )
                    clos.append(mm)
                def tail():
                    outt = out_pool.tile([P, NCH], F32, tag="outp")
                    nc.vector.tensor_copy(out=outt[:], in_=hold["ps"][:])
                    nc.sync.dma_start(
                        out=out_d.ap()[ct * P:(ct + 1) * P,
                                       tc_ * NCH:(tc_ + 1) * NCH],
                        in_=outt[:])
                clos.append(tail)
                return clos

            g7dest = lambda r, qc: ot_all[:, 28 + r, qc * NCH:(qc + 1) * NCH]

            bounce(0)
            bounce(1)
            wo_slab = [load_head_slab(wo_d, ct) for ct in range(3)]
            ch0 = [make_wo_chain(wo_slab[ct], ct, 0) for ct in range(3)]
            emit_unit(7, 0, 0, qT_cur, [], direct_dest=g7dest(0, 0))
            emit_unit(7, 1, 0, qT_cur, ch0[0][0:7], direct_dest=g7dest(1, 0))
            emit_unit(7, 2, 0, qT_cur, ch0[1][0:7], direct_dest=g7dest(2, 0))
            emit_unit(7, 3, 0, qT_cur, ch0[2][0:7], direct_dest=g7dest(3, 0))
            for ct in range(3):
                for cl in ch0[ct][7:]:
                    cl()
            ch1 = [make_wo_chain(wo_slab[ct], ct, 1) for ct in range(3)]
            emit_unit(7, 0, 1, qT_cur, ch1[0][0:7], direct_dest=g7dest(0, 1))
            emit_unit(7, 1, 1, qT_cur, ch1[1][0:7], direct_dest=g7dest(1, 1))
            emit_unit(7, 2, 1, qT_cur, ch1[2][0:7], direct_dest=g7dest(2, 1))
            emit_unit(7, 3, 1, qT_cur, [], direct_dest=g7dest(3, 1))
            for ct in range(3):
                for cl in ch1[ct][7:]:
                    cl()
            for ct in range(3, KC):
                slab = load_head_slab(wo_d, ct)
                for tc_ in range(TC):
                    for cl in make_wo_chain(slab, ct, tc_):
                        cl()
            ores_cm.__exit__(None, None, None)
            kvres_cm.__exit__(None, None, None)

    nc.compile()
    return nc


# host-side input preparation -------------------------------------------------

_ROPE_PERM = np.concatenate([np.arange(0, HEAD_DIM, 2), np.arange(1, HEAD_DIM, 2)])


def _permute_heads(w, n_heads):
    """Permute columns within each head so rotation pairs become
    contiguous (even | odd) halves."""
    w = w.reshape(w.shape[0], n_heads, HEAD_DIM)
    return w[:, :, _ROPE_PERM].reshape(w.shape[0], n_heads * HEAD_DIM)


def _w_layout(w):
    """[DIM, C] f32 -> [C/512, 2, 128, 16, 512] bf16 moving-slab layout."""
    C = w.shape[1]
    wl = w.reshape(2, KH, P, C // NCH, NCH).transpose(3, 0, 2, 1, 4)
    return np.ascontiguousarray(wl).astype(ml_dtypes.bfloat16)


def _prep_shared(cos, sin, wq, wk, wv, wo):
    wq_p = _permute_heads(np.asarray(wq, dtype=np.float32), N_HEADS)
    wk_p = _permute_heads(np.asarray(wk, dtype=np.float32), N_KV)
    # lhsT chunk layouts: [g, h, k-part, kc, d] / [h, k-part, kc, d]
    wq_l = np.ascontiguousarray(
        wq_p.reshape(KC, P, N_KV, REP, HEAD_DIM).transpose(2, 3, 1, 0, 4)
    ).astype(ml_dtypes.bfloat16)
    wk_l = np.ascontiguousarray(
        wk_p.reshape(KC, P, N_KV, HEAD_DIM).transpose(2, 1, 0, 3)
    ).astype(ml_dtypes.bfloat16)
    wv_l = _w_layout(np.asarray(wv, dtype=np.float32))
    # wo lhsT slabs [ct, d, h, c]
    wo_l = np.ascontiguousarray(
        np.asarray(wo, dtype=np.float32)
        .reshape(N_HEADS, HEAD_DIM, KC, P).transpose(2, 1, 0, 3)
    ).astype(ml_dtypes.bfloat16)
    # positions restart at 0 per block, so block 0's tables serve all cores
    c64 = np.asarray(cos[:S], dtype=np.float32).T          # [64, S]
    s64 = np.asarray(sin[:S], dtype=np.float32).T
    c2_l = np.ascontiguousarray(
        np.concatenate([c64, c64], axis=0)).astype(ml_dtypes.bfloat16)
    s2n_l = np.ascontiguousarray(
        np.concatenate([-s64, s64], axis=0)).astype(ml_dtypes.bfloat16)
    return c2_l, s2n_l, wq_l, wk_l, wv_l, wo_l


def _prep_x_block(xb):
    """x block [S, DIM] f32 -> xt [TT, 128, KC, 128] bf16 (transposed,
    slice-major)."""
    xtb = xb.T.reshape(KC, P, TT, P).transpose(2, 1, 0, 3)
    return np.ascontiguousarray(xtb).astype(ml_dtypes.bfloat16)


def kernel(x, cos, sin, wq, wk, wv, wo):
    if "nc" not in _CACHE:
        _CACHE["nc"] = build_nc()
    nc = _CACHE["nc"]

    x = np.asarray(x, dtype=np.float32)
    c2_l, s2n_l, wq_l, wk_l, wv_l, wo_l = _prep_shared(cos, sin, wq, wk, wv, wo)

    in_maps = []
    for b in range(B):
        in_maps.append({
            "xt": _prep_x_block(x[b * S:(b + 1) * S]),
            "c2": c2_l,
            "s2n": s2n_l,
            "wq": wq_l,
            "wk": wk_l,
            "wv": wv_l,
            "wo": wo_l,
        })
    _CACHE["last_in_maps"] = in_maps
    res = run_bass_kernel_spmd(nc, in_maps, core_ids=list(range(B)))
    _CACHE["last_results"] = res
    # per-core output is transposed [DIM, S]
    out = np.concatenate([res.results[b]["out"].T for b in range(B)], axis=0)
    return np.ascontiguousarray(out, dtype=np.float32)


# revision 36
# speedup vs baseline: 1.1396x; 1.0016x over previous
"""Trainium2 Bass kernel for nn_Attention_45999099740384.

GQA attention over 8 independent packed sequences (block-diagonal mask with
equal blocks). Sharding: data-parallel over the 8 blocks - one block of
S=1024 tokens per NeuronCore, weights replicated, zero collectives.

Per-core pipeline (bf16 TensorEngine, fp32 PSUM):
  1. v projection in natural layout (xt stationary, wv moving).
  2. q/k projections TRANSPOSED (weight chunks stationary, xt moving) so
     q^T/k^T come out in [head_dim, t] layout directly - no PE transposes.
  3. RoPE applied in transposed layout on VectorE: host-permuted weight
     columns put rotation pairs into (even|odd) partition halves; the
     cross-partition half-swap is done with two 64-partition copies, then
     two mults against duplicated cos / sign-flipped sin tables and an add.
  4. scores computed transposed: ST[s,q] = kT.T @ qT -> ScalarE exp ->
     P^T tiles in SBUF; P@V needs no transpose of P.
  5. softmax row-sums via a ones[128,128] stationary matmul; reciprocal on
     VectorE (keeps ScalarE exp-only: a single activation table, no
     ACT_TABLE_LOAD churn); normalization deferred to after P@V.
  6. wo matmul from the transposed attention output.

Scheduling: the q projection of group g+1 is woven between the score
matmuls of group g's attention units so the PE never waits for ScalarE
exp; for the last group the leading wo matmul chains are woven in instead.
Attention outputs of groups 0-6 bounce through DRAM (SBUF cannot hold
ot_all while xt is still resident); group 7 writes ot_all directly.
"""

import numpy as np
import ml_dtypes

import concourse.bass as bass
import concourse.mybir as mybir
import concourse.tile as tile
from concourse import bacc
from concourse.bass_utils import run_bass_kernel_spmd

# problem constants (hardcoded per task instructions)
DIM = 4096
N_HEADS = 32
HEAD_DIM = 128
N_KV = 8
REP = 4
B = 8
S = 1024
T = B * S

P = 128                  # SBUF partitions
KC = DIM // P            # 32 contraction chunks of 128
KH = KC // 2             # 16 (w_layout half-chunk count)
TT = S // P              # 8 token tiles of 128
TC = S // 512            # 2 token chunks of 512
NCH = 512                # matmul moving free dim
SCALE = HEAD_DIM ** -0.5

F32 = mybir.dt.float32
BF16 = mybir.dt.bfloat16

_CACHE = {}


def build_nc():
    nc = bacc.Bacc("TRN2", target_bir_lowering=False, debug=False, num_devices=8)

    # xt slice-major: [t-slice, k-part, kc, t-within-slice] so slice DMAs are
    # contiguous 8KB/partition (strided layout produced 256B DMA packets)
    xt_d = nc.dram_tensor("xt", [TT, P, KC, P], BF16, kind="ExternalInput")
    c2_d = nc.dram_tensor("c2", [P, S], BF16, kind="ExternalInput")
    s2n_d = nc.dram_tensor("s2n", [P, S], BF16, kind="ExternalInput")
    # wq: [g, h, k-part, kc, d] lhsT chunks (rope-permuted d columns)
    wq_d = nc.dram_tensor("wq", [N_KV, REP, P, KC, HEAD_DIM], BF16,
                          kind="ExternalInput")
    wk_d = nc.dram_tensor("wk", [N_KV, P, KC, HEAD_DIM], BF16,
                          kind="ExternalInput")
    # wv/wo: moving-operand slabs [chunk, half, 128, 16, 512]
    wv_d = nc.dram_tensor("wv", [2, 2, P, KH, NCH], BF16, kind="ExternalInput")
    # wo: [ct, d, h, c] lhsT slabs (stationary, streamed per 128-col tile)
    wo_d = nc.dram_tensor("wo", [KC, P, N_HEADS, P], BF16, kind="ExternalInput")
    # transposed output [DIM, S]; host untransposes
    out_d = nc.dram_tensor("out", [DIM, S], F32, kind="ExternalOutput")
    # attention-output bounce for groups 0..6
    otb_d = nc.dram_tensor("otb", [2, P, 28, NCH], BF16)

    with tile.TileContext(nc) as tc:
        with (
            tc.tile_pool(name="const", bufs=1) as const,
            tc.tile_pool(name="wpool", bufs=1) as wpool,
            tc.tile_pool(name="qtg", bufs=2) as qtg_pool,
            tc.tile_pool(name="ptp", bufs=2) as pt_pool,
            tc.tile_pool(name="scr", bufs=3) as scr_pool,
            tc.tile_pool(name="rcbp", bufs=1) as rcb_pool,
            tc.tile_pool(name="otp", bufs=1) as ot_pool,
            tc.tile_pool(name="outp", bufs=2) as out_pool,
            tc.tile_pool(name="pt8p", bufs=1) as pt8_pool,
            tc.tile_pool(name="psmm", bufs=2, space="PSUM") as ps_pool,
            tc.tile_pool(name="psst", bufs=3, space="PSUM") as st_pool,
            tc.tile_pool(name="psor", bufs=3, space="PSUM") as or_pool,
        ):
            ones8 = const.tile([P, 2, P], mybir.dt.float8e5)
            nc.vector.memset(ones8[:], 1.0)
            # exp bias -ln(128): keeps exp within fp8e4 range for the fp8
            # row-sum copy; cancels in the softmax normalization
            ebias = const.tile([P, 1], F32)
            nc.vector.memset(ebias[:], -4.852030263919617)

            kvres_cm = tc.tile_pool(name="kvres", bufs=1)
            kvres = kvres_cm.__enter__()
            kT = kvres.tile([P, N_KV, S], BF16)              # [d, kv, t]
            vN = kvres.tile([P, TT, N_KV * HEAD_DIM], BF16)  # [s, s_tile, kv*d]

            xres_cm = tc.tile_pool(name="xres", bufs=1)
            xres = xres_cm.__enter__()
            xt = xres.tile([P, TT, KC, P], BF16)  # [k-part, slice, kc, tj]

            # ---- weight streaming helpers --------------------------------
            def load_head_slab(w_dram, idx):
                """[P, KC, 128] lhsT slab for one q/k head (2 DMAs)."""
                sl = wpool.tile([P, KC, HEAD_DIM], BF16, tag="wst", bufs=3)
                src = w_dram.ap()[idx] if isinstance(idx, int) \
                    else w_dram.ap()[idx[0], idx[1]]
                nc.sync.dma_start(out=sl[:, 0:KH, :], in_=src[:, 0:KH, :])
                nc.sync.dma_start(out=sl[:, KH:KC, :], in_=src[:, KH:KC, :])
                return sl

            # ---- startup DMAs (xt slices interleaved with wv slabs) ------
            def load_sixteenth(w_dram, cc, e):
                sl = wpool.tile([P, 2, NCH], BF16, tag="w16", bufs=17,
                                name="w16t")
                nc.sync.dma_start(
                    out=sl[:],
                    in_=w_dram.ap()[cc, e // 8, :,
                                    (e % 8) * 2:(e % 8) * 2 + 2, :])
                return sl

            def xslice(ts):
                nc.sync.dma_start(out=xt[:, ts, 0:KH, :],
                                  in_=xt_d.ap()[ts, :, 0:KH, :])
                nc.sync.dma_start(out=xt[:, ts, KH:KC, :],
                                  in_=xt_d.ap()[ts, :, KH:KC, :])

            wv_e = [[], []]
            wv_e[0].append(load_sixteenth(wv_d, 0, 0))
            # first slice in quarters so the leading projection matmuls
            # unblock as early as possible
            for kq in range(4):
                nc.sync.dma_start(
                    out=xt[:, 0, kq * 8:(kq + 1) * 8, :],
                    in_=xt_d.ap()[0, :, kq * 8:(kq + 1) * 8, :])
            for e in range(1, 6):
                wv_e[0].append(load_sixteenth(wv_d, 0, e))
            xslice(1)
            for e in range(6, 11):
                wv_e[0].append(load_sixteenth(wv_d, 0, e))
            xslice(2)
            for e in range(11, 16):
                wv_e[0].append(load_sixteenth(wv_d, 0, e))
            xslice(3)
            for e in range(8):
                wv_e[1].append(load_sixteenth(wv_d, 1, e))
            xslice(4)
            xslice(5)
            for e in range(8, 16):
                wv_e[1].append(load_sixteenth(wv_d, 1, e))
            xslice(6)
            xslice(7)
            c2 = const.tile([P, S], BF16)
            nc.sync.dma_start(out=c2[:], in_=c2_d.ap())
            s2n = const.tile([P, S], BF16)
            nc.sync.dma_start(out=s2n[:], in_=s2n_d.ap())

            # ---- compute helpers -----------------------------------------
            def proj_T_mms(slab, tc_, ps, k0, k1):
                """Transposed projection: out[d, t] += slab[kc].T @ xt."""
                for kc in range(k0, k1):
                    nc.tensor.matmul(
                        ps[:],
                        lhsT=slab[:, kc, :],
                        rhs=xt[:, 4 * tc_:4 * tc_ + 4, kc, :],
                        start=(kc == 0),
                        stop=(kc == KC - 1),
                    )

            def rope_t(ps, tc_, dest):
                """RoPE in [d, t] layout: dest = ps*c2 + swap_halves(ps)*s2n."""
                sw = scr_pool.tile([P, NCH], F32, tag="scr")
                nc.vector.tensor_copy(out=sw[0:64, :], in_=ps[64:P, :])
                nc.vector.tensor_copy(out=sw[64:P, :], in_=ps[0:64, :])
                m1 = scr_pool.tile([P, NCH], F32, tag="scr")
                nc.vector.tensor_tensor(m1[:], ps[:],
                                        c2[:, tc_ * NCH:(tc_ + 1) * NCH],
                                        mybir.AluOpType.mult)
                m2 = scr_pool.tile([P, NCH], F32, tag="scr")
                nc.vector.tensor_tensor(m2[:], sw[:],
                                        s2n[:, tc_ * NCH:(tc_ + 1) * NCH],
                                        mybir.AluOpType.mult)
                nc.vector.tensor_tensor(dest, m1[:], m2[:],
                                        mybir.AluOpType.add)

            def make_proj_filler(slab, tc_, dest):
                """Closures: 8x(4 proj matmuls) + rope. First closure
                allocates the psum tile."""
                hold = {}
                clos = []
                for j in range(8):
                    def mm(j=j):
                        if j == 0:
                            hold["ps"] = ps_pool.tile([P, NCH], F32, tag="mm",
                                                      name="pjps")
                        proj_T_mms(slab, tc_, hold["ps"], j * 4, (j + 1) * 4)
                    clos.append(mm)
                def rope():
                    rope_t(hold["ps"], tc_, dest)
                clos.append(rope)
                return clos

            def emit_unit(g, r, qc, qT_use, filler, direct_dest=None):
                """One attention unit (kv-group g, q-head r, q-col chunk qc)
                with PE filler closures woven between score matmuls."""
                pt = pt_pool.tile([P, TT, NCH], BF16, tag="pt")

                def score(st):
                    sps = st_pool.tile([P, NCH], F32, tag="st")
                    nc.tensor.matmul(
                        sps[:],
                        lhsT=kT[:, g, st * P:(st + 1) * P],
                        rhs=qT_use[:, r, qc * NCH:(qc + 1) * NCH],
                        start=True, stop=True,
                    )
                    nc.scalar.activation(
                        pt[:, st, :], sps[:],
                        mybir.ActivationFunctionType.Exp, scale=SCALE,
                        bias=ebias[:],
                    )

                fi = 0
                score(0)
                score(1)
                for st in range(2, TT):
                    if fi < len(filler):
                        filler[fi]()
                        fi += 1
                    score(st)
                # fp8 copy of P^T for the double-row row-sum matmuls
                pt8 = pt8_pool.tile([P, TT, NCH], mybir.dt.float8e5, tag="pt8")
                nc.vector.tensor_copy(out=pt8[:], in_=pt[:])
                while fi < len(filler):
                    filler[fi]()
                    fi += 1

                ops = or_pool.tile([P, NCH], F32, tag="or")
                rps = or_pool.tile([P, NCH], F32, tag="or")
                for st in range(TT):
                    nc.tensor.matmul(
                        ops[:], lhsT=vN[:, st, g * P:(g + 1) * P],
                        rhs=pt[:, st, :], start=(st == 0), stop=(st == TT - 1),
                    )
                for j in range(TT // 2):
                    nc.tensor.matmul(
                        rps[:], lhsT=ones8[:],
                        rhs=pt8[:, 2 * j:2 * j + 2, :],
                        start=(j == 0), stop=(j == TT // 2 - 1),
                        perf_mode=mybir.MatmulPerfMode.DoubleRow,
                    )
                rcb = rcb_pool.tile([P, NCH], F32, tag="rcb")
                nc.vector.reciprocal_approx_fast(out=rcb[:], in_=rps[:])
                if direct_dest is not None:
                    nc.vector.tensor_tensor(direct_dest, ops[:], rcb[:],
                                            mybir.AluOpType.mult)
                else:
                    ot = ot_pool.tile([P, NCH], BF16, tag="ot")
                    nc.vector.tensor_tensor(ot[:], ops[:], rcb[:],
                                            mybir.AluOpType.mult)
                    nc.sync.dma_start(out=otb_d.ap()[qc, :, g * REP + r, :],
                                      in_=ot[:])

            # ---- phase 1: v projection (natural layout) ------------------
            for cc in range(2):
                sixteenths = wv_e[cc]
                for tt in range(TT):
                    ps = ps_pool.tile([P, NCH], F32, tag="mm")
                    for kc in range(KC):
                        nc.tensor.matmul(
                            ps[:],
                            lhsT=xt[:, tt, kc, :],
                            rhs=sixteenths[kc // 2][:, kc % 2, :],
                            start=(kc == 0), stop=(kc == KC - 1),
                        )
                    nc.vector.tensor_copy(
                        out=vN[:, tt, cc * NCH:(cc + 1) * NCH], in_=ps[:])

            # queue q/k head slabs in consumption order (ring-gated)
            wk_slab = [load_head_slab(wk_d, h) for h in range(N_KV)]
            wq_slab = {}
            for h in range(REP):
                wq_slab[(0, h)] = load_head_slab(wq_d, (0, h))
            for h in range(REP):
                wq_slab[(1, h)] = load_head_slab(wq_d, (1, h))

            # ---- phase 2: k projection (transposed) + RoPE ---------------
            for h in range(N_KV):
                for tc_ in range(TC):
                    ps = ps_pool.tile([P, NCH], F32, tag="mm")
                    proj_T_mms(wk_slab[h], tc_, ps, 0, KC)
                    rope_t(ps, tc_, kT[:, h, tc_ * NCH:(tc_ + 1) * NCH])

            # ---- phase 3: q projection for group 0 -----------------------
            qT_cur = qtg_pool.tile([P, REP, S], BF16, tag="qtg")
            for h in range(REP):
                for tc_ in range(TC):
                    ps = ps_pool.tile([P, NCH], F32, tag="mm")
                    proj_T_mms(wq_slab[(0, h)], tc_, ps, 0, KC)
                    rope_t(ps, tc_, qT_cur[:, h, tc_ * NCH:(tc_ + 1) * NCH])

            # ---- groups 0..6: attention woven with next q projection -----
            for g in range(N_KV - 1):
                qT_next = qtg_pool.tile([P, REP, S], BF16, tag="qtg")
                fillers = [
                    make_proj_filler(
                        wq_slab[(g + 1, i // 2)], i % 2,
                        qT_next[:, i // 2, (i % 2) * NCH:(i % 2 + 1) * NCH])
                    for i in range(8)
                ]
                for i in range(8):
                    if g + 2 <= N_KV - 1 and i in (2, 4, 6, 7):
                        hh = {2: 0, 4: 1, 6: 2, 7: 3}[i]
                        wq_slab[(g + 2, hh)] = load_head_slab(wq_d, (g + 2, hh))
                    emit_unit(g, i // 2, i % 2, qT_cur, fillers[i])
                qT_cur = qT_next

            # ---- group 7 + wo --------------------------------------------
            xres_cm.__exit__(None, None, None)
            ores_cm = tc.tile_pool(name="ores", bufs=1)
            ores = ores_cm.__enter__()
            ot_all = ores.tile([P, N_HEADS, S], BF16)  # [d, h, t]

            def bounce(qc):
                for g in range(7):
                    nc.sync.dma_start(
                        out=ot_all[:, g * REP:(g + 1) * REP,
                                   qc * NCH:(qc + 1) * NCH],
                        in_=otb_d.ap()[qc, :, g * REP:(g + 1) * REP, :],
                    )

            def make_wo_chain(slab, ct, tc_):
                """Closures: 8x(4 wo matmuls over heads, slab stationary)
                + copy/DMA tail writing the transposed output."""
                hold = {}
                clos = []
                for j in range(8):
                    def mm(j=j):
                        if j == 0:
                            hold["ps"] = ps_pool.tile([P, NCH], F32, tag="mm",
                                                      name="wops")
                        for h in range(j * 4, (j + 1) * 4):
                            nc.tensor.matmul(
                                hold["ps"][:],
                                lhsT=slab[:, h, :],
                                rhs=ot_all[:, h, tc_ * NCH:(tc_ + 1) * NCH],
                                start=(h == 0), stop=(h == N_HEADS - 1),
                            )
                    clos.append(mm)
                def tail():
                    outt = out_pool.tile([P, NCH], F32, tag="outp")
                    nc.vector.tensor_copy(out=outt[:], in_=hold["ps"][:])
                    nc.sync.dma_start(
                        out=out_d.ap()[ct * P:(ct + 1) * P,
                                       tc_ * NCH:(tc_ + 1) * NCH],
                        in_=outt[:])
                clos.append(tail)
                return clos

            g7dest = lambda r, qc: ot_all[:, 28 + r, qc * NCH:(qc + 1) * NCH]

            bounce(0)
            bounce(1)
            wo_slab = [load_head_slab(wo_d, ct) for ct in range(3)]
            ch0 = [make_wo_chain(wo_slab[ct], ct, 0) for ct in range(3)]
            emit_unit(7, 0, 0, qT_cur, [], direct_dest=g7dest(0, 0))
            emit_unit(7, 1, 0, qT_cur, ch0[0][0:7], direct_dest=g7dest(1, 0))
            emit_unit(7, 2, 0, qT_cur, ch0[1][0:7], direct_dest=g7dest(2, 0))
            emit_unit(7, 3, 0, qT_cur, ch0[2][0:7], direct_dest=g7dest(3, 0))
            for ct in range(3):
                for cl in ch0[ct][7:]:
                    cl()
            ch1 = [make_wo_chain(wo_slab[ct], ct, 1) for ct in range(3)]
            emit_unit(7, 0, 1, qT_cur, ch1[0][0:7], direct_dest=g7dest(0, 1))
            emit_unit(7, 1, 1, qT_cur, ch1[1][0:7], direct_dest=g7dest(1, 1))
            emit_unit(7, 2, 1, qT_cur, ch1[2][0:7], direct_dest=g7dest(2, 1))
            emit_unit(7, 3, 1, qT_cur, [], direct_dest=g7dest(3, 1))
            for ct in range(3):
                for cl in ch1[ct][7:]:
                    cl()
            for ct in range(3, KC):
                slab = load_head_slab(wo_d, ct)
                for tc_ in range(TC):
                    for cl in make_wo_chain(slab, ct, tc_):
                        cl()
            ores_cm.__exit__(None, None, None)
            kvres_cm.__exit__(None, None, None)

    nc.compile()
    return nc


# host-side input preparation -------------------------------------------------

_ROPE_PERM = np.concatenate([np.arange(0, HEAD_DIM, 2), np.arange(1, HEAD_DIM, 2)])


def _permute_heads(w, n_heads):
    """Permute columns within each head so rotation pairs become
    contiguous (even | odd) halves."""
    w = w.reshape(w.shape[0], n_heads, HEAD_DIM)
    return w[:, :, _ROPE_PERM].reshape(w.shape[0], n_heads * HEAD_DIM)


def _w_layout(w):
    """[DIM, C] f32 -> [C/512, 2, 128, 16, 512] bf16 moving-slab layout."""
    C = w.shape[1]
    wl = w.reshape(2, KH, P, C // NCH, NCH).transpose(3, 0, 2, 1, 4)
    return np.ascontiguousarray(wl).astype(ml_dtypes.bfloat16)


def _prep_shared(cos, sin, wq, wk, wv, wo):
    wq_p = _permute_heads(np.asarray(wq, dtype=np.float32), N_HEADS)
    wk_p = _permute_heads(np.asarray(wk, dtype=np.float32), N_KV)
    # lhsT chunk layouts: [g, h, k-part, kc, d] / [h, k-part, kc, d]
    wq_l = np.ascontiguousarray(
        wq_p.reshape(KC, P, N_KV, REP, HEAD_DIM).transpose(2, 3, 1, 0, 4)
    ).astype(ml_dtypes.bfloat16)
    wk_l = np.ascontiguousarray(
        wk_p.reshape(KC, P, N_KV, HEAD_DIM).transpose(2, 1, 0, 3)
    ).astype(ml_dtypes.bfloat16)
    wv_l = _w_layout(np.asarray(wv, dtype=np.float32))
    # wo lhsT slabs [ct, d, h, c]
    wo_l = np.ascontiguousarray(
        np.asarray(wo, dtype=np.float32)
        .reshape(N_HEADS, HEAD_DIM, KC, P).transpose(2, 1, 0, 3)
    ).astype(ml_dtypes.bfloat16)
    # positions restart at 0 per block, so block 0's tables serve all cores
    c64 = np.asarray(cos[:S], dtype=np.float32).T          # [64, S]
    s64 = np.asarray(sin[:S], dtype=np.float32).T
    c2_l = np.ascontiguousarray(
        np.concatenate([c64, c64], axis=0)).astype(ml_dtypes.bfloat16)
    s2n_l = np.ascontiguousarray(
        np.concatenate([-s64, s64], axis=0)).astype(ml_dtypes.bfloat16)
    return c2_l, s2n_l, wq_l, wk_l, wv_l, wo_l


def _prep_x_block(xb):
    """x block [S, DIM] f32 -> xt [TT, 128, KC, 128] bf16 (transposed,
    slice-major)."""
    xtb = xb.T.reshape(KC, P, TT, P).transpose(2, 1, 0, 3)
    return np.ascontiguousarray(xtb).astype(ml_dtypes.bfloat16)


def kernel(x, cos, sin, wq, wk, wv, wo):
    if "nc" not in _CACHE:
        _CACHE["nc"] = build_nc()
    nc = _CACHE["nc"]

    x = np.asarray(x, dtype=np.float32)
    c2_l, s2n_l, wq_l, wk_l, wv_l, wo_l = _prep_shared(cos, sin, wq, wk, wv, wo)

    in_maps = []
    for b in range(B):
        in_maps.append({
            "xt": _prep_x_block(x[b * S:(b + 1) * S]),
            "c2": c2_l,
            "s2n": s2n_l,
            "wq": wq_l,
            "wk": wk_l,
            "wv": wv_l,
            "wo": wo_l,
        })
    _CACHE["last_in_maps"] = in_maps
    res = run_bass_kernel_spmd(nc, in_maps, core_ids=list(range(B)))
    _CACHE["last_results"] = res
    # per-core output is transposed [DIM, S]
    out = np.concatenate([res.results[b]["out"].T for b in range(B)], axis=0)
    return np.ascontiguousarray(out, dtype=np.float32)
